# revision 7
# baseline (speedup 1.0000x reference)
"""Self-contained Trainium2 Bass kernel for nn_GCNAutoencoder_4827543241244.

Replicates reference.py's exact semantics (including jax OOB behavior: scatter
drops, gather clips — see analysis below), distributed over 8 NeuronCores.

True computation (from reference.py with jax OOB semantics):
  nodes 0..119999; member-node k = node k (feature x_member[k]);
  prov-node k = node 100000+k (feature x_provider[k] zero-padded to 128).
  For each input edge e (p_e=provider_idx[e]<20000, m_e=member_idx[e]<100000):
    B-edge (always):          prov-node min(m_e,19999) -> member-node p_e
    A-edge (only m_e<20000):  member-node p_e          -> prov-node m_e
  deg[member-node k<20000] = 1+|{e:p_e=k}|; deg[member-node k>=20000] = 1;
  deg[prov-node k] = 1+|{e:m_e=k, m_e<20000}|... = 1+cnt_m(k).
  GCN layer: out[d] = dinv_d*sum_e dinv_s*(x_s@W) + dinv_d^2*(x_d@W) + b.
  h = relu(out1); z = gcn2(h); x_hat = z@Wdec + bdec.
  Member-nodes >= 20000 are a pure per-row MLP.

Device strategy per core (8-way SPMD):
  - shards: active members [c*2500,(c+1)*2500), prov-nodes likewise -> 40 dst
    blocks of 128; passive members 10000 rows/core (MLP).
  - Segment-sum on TensorE: per 128-edge chunk, gather source rows (dma_gather,
    bf16 256B rows) as stationary lhsT; one-hot S (built by one DVE
    tensor_scalar: (iota==dst_local)*w_src) as moving rhs; PSUM accumulates
    segsum^T [feat, dst]. Self-loop term via a diagonal matmul that also
    initializes PSUM (start=True). All dst-side dinv scaling is deferred (relu
    commutes with positive col-scales; biases handled via K=1 matmuls against
    sqrt(deg)) and applied once at the end.
  - Layer2 shares z'=hpre@W2 (padded bf16 rows, garbage pad cols) across cores
    via two AllGathers; gathers read the AG output directly.
"""
import numpy as np
import ml_dtypes

import concourse.bass as bass
import concourse.bacc as bacc
import concourse.mybir as mybir
import concourse.tile as tile
from concourse.bass_utils import run_bass_kernel_spmd

bfnp = ml_dtypes.bfloat16
BF = mybir.dt.bfloat16
F32 = mybir.dt.float32
I16 = mybir.dt.int16

NM, NP_, E = 100000, 20000, 500000
DM, DP, HID, LAT = 128, 64, 128, 32
NCORES = 8
NACT = 20000
ASH = NACT // NCORES          # 2500
ABLK = 20                     # 128-blocks per shard side
APAD = ABLK * 128             # 2560
NBLK = 2 * ABLK               # 40
SHROWS = 2 * APAD             # 5120
NPASS = NM - NACT             # 80000
PSH = NPASS // NCORES         # 10000
PASSW = 10240                 # padded passive cols per core
MAXCH = 32                    # chunks per dma_gather call (4096 idx)

WIN_B = [(0, 32), (32, 32), (64, 32), (96, 32)]
WIN_A = [(0, 64), (64, 64)]


def _tab2_row(k):
    return (k // ASH) * APAD + (k % ASH)


def _build_sched(per_core, nblocks, windows):
    """per_core: list of 8 (dst_local, src_row, w_src). Returns uniform schedule
    + per-core filled arrays (sr, dl, ws) of shape [nch*128]."""
    nw = len(windows)
    wb_arr = np.array([w[0] for w in windows])
    counts = np.zeros((NCORES, nblocks, nw), np.int64)
    parts = {}
    for c, (dl, sr, ws) in enumerate(per_core):
        blk = dl // 128
        dlb = dl - blk * 128
        wi = np.searchsorted(wb_arr, dlb, side="right") - 1
        key = blk * nw + wi
        order = np.lexsort((dl, key))
        parts[c] = (key[order], dl[order], sr[order], ws[order])
        np.add.at(counts[c], (blk, wi), 1)
    budget = -(-counts.max(axis=0) // 128)          # [nblocks, nw] chunks
    sched = []                                       # (block, wb, ww, nk)
    for b in range(nblocks):
        for w in range(nw):
            if budget[b, w]:
                sched.append((b, windows[w][0], windows[w][1], int(budget[b, w])))
    nch = int(budget.sum())
    core_arrs = []
    for c in range(NCORES):
        key, dl, sr, ws = parts[c]
        DLa = np.full(nch * 128, -1, np.float32)
        SRa = np.zeros(nch * 128, np.int64)
        WSa = np.zeros(nch * 128, np.float32)
        pos = 0
        i = 0
        for b in range(nblocks):
            for w in range(nw):
                nk = budget[b, w]
                if nk == 0:
                    continue
                kk = b * nw + w
                j = i
                while j < len(key) and key[j] == kk:
                    j += 1
                n = j - i
                assert n <= nk * 128
                DLa[pos:pos + n] = dl[i:j] - b * 128
                SRa[pos:pos + n] = sr[i:j]
                WSa[pos:pos + n] = ws[i:j]
                pos += nk * 128
                i = j
        core_arrs.append((DLa, SRa, WSa))
    return sched, nch, core_arrs


def _pack_idx(sr):
    """[nch*128] -> wrapped int16 idx array [128, nch*8]."""
    n = len(sr)
    arr = np.zeros((16, n // 16), np.int16)
    ii = np.arange(n)
    arr[ii % 16, ii // 16] = sr.astype(np.int16)
    return np.tile(arr, (8, 1))


def _preprocess(provider_idx, member_idx):
    p_e = np.asarray(provider_idx).astype(np.int64)
    m_e = np.asarray(member_idx).astype(np.int64)
    cnt_p = np.bincount(p_e, minlength=NACT)[:NACT]
    cnt_m = np.bincount(np.minimum(m_e, NACT), minlength=NACT + 1)[:NACT]
    # deg[prov-node k] counts edges with m_e == k (k<20000) -- min() above would
    # wrongly add m_e>=20000 edges to node 19999; recount properly:
    cnt_m = np.bincount(m_e[m_e < NACT], minlength=NACT)[:NACT]
    dinv_mn = (1.0 + cnt_p) ** -0.5
    dinv_pn = (1.0 + cnt_m) ** -0.5

    mclip = np.minimum(m_e, NACT - 1)
    a_mask = m_e < NACT

    coreB, coreA = {1: [], 2: []}, {1: [], 2: []}
    for c in range(NCORES):
        lo, hi = c * ASH, (c + 1) * ASH
        sb = (p_e >= lo) & (p_e < hi)
        coreB[1].append((p_e[sb] - lo, mclip[sb], dinv_pn[mclip[sb]]))
        coreB[2].append((p_e[sb] - lo, _tab2_row(mclip[sb]), dinv_pn[mclip[sb]] ** 2))
        sa = a_mask & (m_e >= lo) & (m_e < hi)
        coreA[1].append((m_e[sa] - lo, p_e[sa], dinv_mn[p_e[sa]]))
        coreA[2].append((m_e[sa] - lo, _tab2_row(p_e[sa]), dinv_mn[p_e[sa]] ** 2))

    streams = {}
    for L in (1, 2):
        streams[f"B{L}"] = _build_sched(coreB[L], ABLK, WIN_B)
        streams[f"A{L}"] = _build_sched(coreA[L], ABLK, WIN_A)

    per_core_meta = []
    for c in range(NCORES):
        dd = np.zeros(SHROWS, np.float64)
        dd[:ASH] = dinv_mn[c * ASH:(c + 1) * ASH]
        dd[APAD:APAD + ASH] = dinv_pn[c * ASH:(c + 1) * ASH]
        sq = np.zeros(SHROWS, np.float64)
        sq[:ASH] = 1.0 / dinv_mn[c * ASH:(c + 1) * ASH]
        sq[APAD:APAD + ASH] = 1.0 / dinv_pn[c * ASH:(c + 1) * ASH]
        per_core_meta.append(dict(
            wdst1=dd.reshape(NBLK, 128).T.astype(np.float32).copy(),
            wdst2=(dd ** 2).reshape(NBLK, 128).T.astype(np.float32).copy(),
            dinv_row=dd.astype(np.float32)[None, :].copy(),
            sqrt_row=sq.astype(np.float32)[None, :].copy(),
        ))
    return streams, per_core_meta


def _emit_stream_consts(nc, name, nch):
    """DRAM tensors for one stream's per-chunk metadata + gather indices."""
    if nch == 0:
        return None
    return dict(
        idx=nc.dram_tensor(f"{name}_idx", [128, nch * 8], I16, kind="ExternalInput").ap(),
        dl=nc.dram_tensor(f"{name}_dl", [128, nch], F32, kind="ExternalInput").ap(),
        ws=nc.dram_tensor(f"{name}_ws", [128, nch], F32, kind="ExternalInput").ap(),
    )


def _build_program(streams):
    nc = bacc.Bacc("TRN2", target_bir_lowering=False, debug=False,
                   num_devices=NCORES)
    D = {}
    D["am_tab"] = nc.dram_tensor("am_tab", [NACT, 128], BF, kind="ExternalInput").ap()
    D["pn_tab"] = nc.dram_tensor("pn_tab", [NACT, 128], BF, kind="ExternalInput").ap()
    D["xsh"] = nc.dram_tensor("xsh", [128, NBLK * 128], BF, kind="ExternalInput").ap()
    D["xpassT"] = nc.dram_tensor("xpassT", [128, PASSW], BF, kind="ExternalInput").ap()
    for nm_ in ("W1", "W2", "Wdec", "Wc"):
        shp = [LAT, 128] if nm_ == "Wdec" else [128, LAT] if nm_ == "W2" else [128, 128]
        D[nm_] = nc.dram_tensor(nm_, shp, BF, kind="ExternalInput").ap()
    D["b1"] = nc.dram_tensor("b1", [1, 128], BF, kind="ExternalInput").ap()
    D["cvec"] = nc.dram_tensor("cvec", [1, 128], BF, kind="ExternalInput").ap()
    D["wdst1"] = nc.dram_tensor("wdst1", [128, NBLK], F32, kind="ExternalInput").ap()
    D["wdst2"] = nc.dram_tensor("wdst2", [128, NBLK], F32, kind="ExternalInput").ap()
    D["dinv_row"] = nc.dram_tensor("dinv_row", [1, SHROWS], F32, kind="ExternalInput").ap()
    D["sqrt_row"] = nc.dram_tensor("sqrt_row", [1, SHROWS], F32, kind="ExternalInput").ap()
    for sname in ("B1", "A1", "B2", "A2"):
        D[sname] = _emit_stream_consts(nc, sname, streams[sname][1])
    D["out_act"] = nc.dram_tensor("out_act", [NBLK, 128, 128], F32, kind="ExternalOutput").ap()
    D["out_pass"] = nc.dram_tensor("out_pass", [128, PASSW], F32, kind="ExternalOutput").ap()

    with tile.TileContext(nc) as tc:
        with (
            tc.tile_pool(name="const", bufs=1) as cpool,
            tc.tile_pool(name="meta", bufs=1) as mpool,
            tc.tile_pool(name="gb", bufs=3) as gpool,
            tc.tile_pool(name="st", bufs=4) as spool,
            tc.tile_pool(name="work", bufs=2) as wpool,
            tc.tile_pool(name="zres", bufs=1) as zpool,
            tc.tile_pool(name="ps", bufs=2, space="PSUM") as ps,
            tc.tile_pool(name="ps2", bufs=2, space="PSUM") as ps2,
            tc.tile_pool(name="psmlp", bufs=2, space="PSUM") as psmlp,
            tc.tile_pool(name="dram", bufs=1, space="DRAM") as dpool,
        ):
            # ---------- constants ----------
            iota = cpool.tile([128, 128], F32)
            nc.gpsimd.iota(iota[:], pattern=[[1, 128]], base=0,
                           channel_multiplier=0, allow_small_or_imprecise_dtypes=True)
            pidx = cpool.tile([128, 1], F32)
            nc.gpsimd.iota(pidx[:], pattern=[[0, 1]], base=0,
                           channel_multiplier=1, allow_small_or_imprecise_dtypes=True)
            ones_row = cpool.tile([1, 512], F32)
            nc.vector.memset(ones_row[:], 1.0)
            ones_bf = cpool.tile([1, 512], BF)
            nc.vector.memset(ones_bf[:], 1.0)
            consts = {}
            for nm_ in ("W1", "W2", "Wdec", "Wc", "b1", "cvec"):
                t = cpool.tile(list(D[nm_].shape), BF, name=f"c_{nm_}")
                nc.sync.dma_start(t[:], D[nm_][:])
                consts[nm_] = t
            wdst1 = cpool.tile([128, NBLK], F32)
            nc.sync.dma_start(wdst1[:], D["wdst1"][:])
            wdst2 = cpool.tile([128, NBLK], F32)
            nc.sync.dma_start(wdst2[:], D["wdst2"][:])
            dinv_row = cpool.tile([1, SHROWS], F32)
            nc.sync.dma_start(dinv_row[:], D["dinv_row"][:])
            sqrt_row = cpool.tile([1, SHROWS], F32)
            nc.sync.dma_start(sqrt_row[:], D["sqrt_row"][:])
            sqrt_bf = cpool.tile([1, SHROWS], BF)
            nc.vector.tensor_copy(sqrt_bf[:], sqrt_row[:])

            # resident per-dst-block self rows: x (L1) and z'pad (L2)
            xsh_t = cpool.tile([128, NBLK * 128], BF)   # [dst%128, b*128+feat]
            nc.sync.dma_start(xsh_t[:], D["xsh"][:])
            zself = zpool.tile([128, NBLK * 128], BF)
            nc.gpsimd.memset(zself[:], 0.0)

            # stream metadata resident tiles
            smeta = {}
            for sname in ("B1", "A1", "B2", "A2"):
                if D[sname] is None:
                    continue
                nch = streams[sname][1]
                dlt = mpool.tile([128, nch], F32, name=f"dl_{sname}")
                nc.sync.dma_start(dlt[:], D[sname]["dl"][:])
                wst = mpool.tile([128, nch], F32, name=f"ws_{sname}")
                nc.sync.dma_start(wst[:], D[sname]["ws"][:])
                idxt = mpool.tile([128, nch * 8], I16, name=f"ix_{sname}")
                nc.sync.dma_start(idxt[:], D[sname]["idx"][:])
                smeta[sname] = (dlt, wst, idxt)

            # AG bounce + tables
            ag_in_am = dpool.tile([APAD, 128], BF)
            ag_in_pn = dpool.tile([APAD, 128], BF)
            am_tab2 = dpool.tile([NCORES * APAD, 128], BF, addr_space="Shared")
            pn_tab2 = dpool.tile([NCORES * APAD, 128], BF, addr_space="Shared")

            # ---------- graph layer ----------
            def graph_layer(L, tabB, tabA, wdst, wexp):
                """wexp: function block -> nothing; L=1 computes z', L=2 out."""
                for sname in (f"B{L}", f"A{L}"):
                    pass
                # precompute call partitions per stream
                def calls_of(nch):
                    out = []
                    c0 = 0
                    while c0 < nch:
                        n = min(MAXCH, nch - c0)
                        out.append((c0, n))
                        c0 += n
                    return out

                for side, tabl in (("B", tabB), ("A", tabA)):
                    sname = f"{side}{L}"
                    sched, nch, _ = streams[sname]
                    if nch == 0:
                        continue
                    dlt, wst, idxt = smeta[sname]
                    calls = calls_of(nch)
                    gt = {}
                    # chunk -> (call index, slot)
                    c2call = {}
                    for ci, (c0, n) in enumerate(calls):
                        for k in range(n):
                            c2call[c0 + k] = (ci, k)
                    blk_off = 0 if side == "B" else ABLK
                    # iterate blocks in schedule order
                    chunk_id = 0
                    bcur = -1
                    psum_blk = None
                    gtile = None
                    gci = -1
                    for (b, wb, ww, nk) in sched:
                        gb = b + blk_off
                        if gb != bcur:
                            if bcur >= 0:
                                finish_block(L, bcur, psum_blk)
                            bcur = gb
                            psum_blk = ps.tile([128, 128], F32, name="blkps", tag="blkps")
                            # self-term (also zeroes psum): D = (iota==p)*wdst[:,gb]
                            Dt = spool.tile([128, 128], BF, name="Dt", tag="s")
                            nc.vector.tensor_scalar(
                                out=Dt[:], in0=iota[:], scalar1=pidx[:],
                                scalar2=wdst[:, gb:gb + 1],
                                op0=mybir.AluOpType.is_equal, op1=mybir.AluOpType.mult)
                            selfsrc = xsh_t if L == 1 else zself
                            nc.tensor.matmul(
                                psum_blk[:],
                                selfsrc[:, gb * 128:(gb + 1) * 128],
                                Dt[:], start=True, stop=False, skip_group_check=True)
                        for k in range(nk):
                            ci, slot = c2call[chunk_id]
                            if ci != gci:
                                c0, ncall = calls[ci]
                                gtile = gpool.tile([128, MAXCH, 128], BF,
                                                   name="g", tag=f"g{side}")
                                nc.gpsimd.dma_gather(
                                    out_ap=gtile[:, :ncall, :],
                                    in_ap=tabl[:],
                                    idxs_ap=idxt[:, c0 * 8:(c0 + ncall) * 8],
                                    num_idxs=ncall * 128,
                                    num_idxs_reg=ncall * 128,
                                    elem_size=128,
                                    single_packet=False)
                                gci = ci
                            St = spool.tile([128, 128], BF, name="St", tag="s")
                            nc.vector.tensor_scalar(
                                out=St[:, :ww], in0=iota[:, wb:wb + ww],
                                scalar1=dlt[:, chunk_id:chunk_id + 1],
                                scalar2=wst[:, chunk_id:chunk_id + 1],
                                op0=mybir.AluOpType.is_equal, op1=mybir.AluOpType.mult)
                            nc.tensor.matmul(
                                psum_blk[:, wb:wb + ww],
                                gtile[:, slot, :],
                                St[:, :ww], start=False, stop=False,
                                skip_group_check=True)
                            chunk_id += 1
                    if bcur >= 0:
                        finish_block(L, bcur, psum_blk)

            def finish_block(L, gb, psum_blk):
                sl = slice(gb * 128, (gb + 1) * 128)
                if L == 1:
                    P1 = wpool.tile([128, 128], BF, name="P1", tag="p1")
                    nc.scalar.activation(P1[:], psum_blk[:], mybir.ActivationFunctionType.Copy)
                    ps_h = ps2.tile([128, 128], F32, name="psh", tag="psh")
                    nc.tensor.matmul(ps_h[:], consts["W1"][:], P1[:],
                                     start=True, stop=False, skip_group_check=True)
                    nc.tensor.matmul(ps_h[:], consts["b1"][:], sqrt_bf[:, sl],
                                     start=False, stop=True, skip_group_check=True)
                    hpre = wpool.tile([128, 128], BF, name="hpre", tag="hpre")
                    nc.scalar.activation(hpre[:], ps_h[:],
                                         mybir.ActivationFunctionType.Relu)
                    ps_z = ps2.tile([128, LAT], F32, name="psz", tag="psz")
                    nc.tensor.matmul(ps_z[:], hpre[:], consts["W2"][:],
                                     start=True, stop=True, skip_group_check=True)
                    # write z' into zself pad tile (cols 32:128 stale = garbage ok)
                    nc.scalar.activation(zself[:, gb * 128:gb * 128 + LAT], ps_z[:],
                                         mybir.ActivationFunctionType.Copy)
                    # export padded z' rows to AG input
                    dst = ag_in_am if gb < ABLK else ag_in_pn
                    bb = gb if gb < ABLK else gb - ABLK
                    nc.sync.dma_start(
                        dst[bb * 128:(bb + 1) * 128, :],
                        zself[:, gb * 128:(gb + 1) * 128])
                else:
                    P2 = wpool.tile([LAT, 128], BF, name="P2", tag="p2")
                    nc.scalar.activation(P2[:], psum_blk[:LAT, :], mybir.ActivationFunctionType.Copy)
                    ps_x = ps2.tile([128, 128], F32, name="psx", tag="psh")
                    nc.tensor.matmul(ps_x[:], consts["Wdec"][:], P2[:],
                                     start=True, stop=False, skip_group_check=True)
                    nc.tensor.matmul(ps_x[:], consts["cvec"][:], sqrt_bf[:, sl],
                                     start=False, stop=True, skip_group_check=True)
                    ps_d = ps2.tile([128, 128], F32, name="psd", tag="psz")
                    nc.tensor.matmul(ps_d[:], ones_row[:, :128], dinv_row[:, sl],
                                     start=True, stop=True, skip_group_check=True)
                    dfin = wpool.tile([128, 128], F32, name="dfin", tag="p2")
                    nc.scalar.activation(dfin[:], ps_d[:], mybir.ActivationFunctionType.Copy)
                    xo = wpool.tile([128, 128], F32, name="xo", tag="xo")
                    nc.vector.tensor_tensor(out=xo[:], in0=ps_x[:], in1=dfin[:],
                                            op=mybir.AluOpType.mult)
                    nc.sync.dma_start(D["out_act"][gb], xo[:])

            graph_layer(1, D["pn_tab"], D["am_tab"], wdst1, None)

            # AllGathers (z' tables)
            nc.gpsimd.collective_compute(
                "AllGather", mybir.AluOpType.bypass,
                replica_groups=[list(range(NCORES))],
                ins=[ag_in_am.opt()], outs=[am_tab2.opt()])
            nc.gpsimd.collective_compute(
                "AllGather", mybir.AluOpType.bypass,
                replica_groups=[list(range(NCORES))],
                ins=[ag_in_pn.opt()], outs=[pn_tab2.opt()])

            graph_layer(2, pn_tab2, am_tab2, wdst2, None)

            # ---------- passive MLP ----------
            for t0 in range(0, PASSW, 512):
                xt = wpool.tile([128, 512], BF, name="xt", tag="mlpx")
                nc.sync.dma_start(xt[:], D["xpassT"][:, t0:t0 + 512])
                ph = psmlp.tile([128, 512], F32, name="ph", tag="mlph")
                nc.tensor.matmul(ph[:], consts["W1"][:], xt[:],
                                 start=True, stop=False, skip_group_check=True)
                nc.tensor.matmul(ph[:], consts["b1"][:], ones_bf[:],
                                 start=False, stop=True, skip_group_check=True)
                hh = wpool.tile([128, 512], BF, name="hh", tag="mlpx")
                nc.scalar.activation(hh[:], ph[:],
                                     mybir.ActivationFunctionType.Relu)
                po = psmlp.tile([128, 512], F32, name="po", tag="mlph")
                nc.tensor.matmul(po[:], consts["Wc"][:], hh[:],
                                 start=True, stop=False, skip_group_check=True)
                nc.tensor.matmul(po[:], consts["cvec"][:], ones_bf[:],
                                 start=False, stop=True, skip_group_check=True)
                xop = wpool.tile([128, 512], F32, name="xop", tag="mlpo")
                nc.vector.tensor_copy(xop[:], po[:])
                nc.sync.dma_start(D["out_pass"][:, t0:t0 + 512], xop[:])

    nc.compile()
    return nc


def kernel(x_member, x_provider, provider_idx, member_idx,
           W1, b1, W2, b2, Wdec, bdec):
    x_member = np.asarray(x_member, np.float32)
    x_provider = np.asarray(x_provider, np.float32)
    provider_idx = np.asarray(provider_idx)
    member_idx = np.asarray(member_idx)
    W1 = np.asarray(W1, np.float32); b1 = np.asarray(b1, np.float32)
    W2 = np.asarray(W2, np.float32); b2 = np.asarray(b2, np.float32)
    Wdec = np.asarray(Wdec, np.float32); bdec = np.asarray(bdec, np.float32)

    streams, meta = _preprocess(provider_idx, member_idx)
    nc = _build_program(streams)

    xm_bf = x_member.astype(bfnp)
    xp_bf = np.zeros((NP_, 128), bfnp)
    xp_bf[:, :DP] = x_provider.astype(bfnp)
    Wc = (W2 @ Wdec).astype(bfnp)
    cvec = (b2 @ Wdec + bdec).astype(bfnp)[None, :]
    shared = {
        "am_tab": xm_bf[:NACT].copy(),
        "pn_tab": xp_bf,
        "W1": W1.astype(bfnp), "W2": W2.astype(bfnp),
        "Wdec": Wdec.astype(bfnp), "Wc": Wc,
        "b1": b1.astype(bfnp)[None, :], "cvec": cvec,
    }
    in_maps = []
    for c in range(NCORES):
        m = dict(shared)
        xsh = np.zeros((SHROWS, 128), bfnp)
        xsh[:ASH] = xm_bf[c * ASH:(c + 1) * ASH]
        xsh[APAD:APAD + ASH] = xp_bf[c * ASH:(c + 1) * ASH]
        # device layout: [dst%128, block*128+feat]
        m["xsh"] = np.ascontiguousarray(
            xsh.reshape(NBLK, 128, 128).transpose(1, 0, 2).reshape(128, NBLK * 128))
        xpT = np.zeros((128, PASSW), bfnp)
        xpT[:, :PSH] = xm_bf[NACT + c * PSH:NACT + (c + 1) * PSH].T
        m["xpassT"] = xpT
        for k in ("wdst1", "wdst2", "dinv_row", "sqrt_row"):
            m[k] = meta[c][k]
        for sname in ("B1", "A1", "B2", "A2"):
            sched, nch, core_arrs = streams[sname]
            if nch == 0:
                continue
            DLa, SRa, WSa = core_arrs[c]
            m[f"{sname}_idx"] = _pack_idx(SRa)
            m[f"{sname}_dl"] = DLa.reshape(nch, 128).T.copy()
            m[f"{sname}_ws"] = WSa.reshape(nch, 128).T.copy()
        in_maps.append(m)

    import os
    if os.environ.get("KERNEL_SIM") == "1":
        import concourse.bass_interp as bass_interp
        sim = bass_interp.MultiCoreSim(nc, NCORES, num_workers=1)
        for c in range(NCORES):
            cs = sim.cores[c]
            for k, v in in_maps[c].items():
                cs.tensor(k)[:] = v
        sim.simulate()
        class _R:
            pass
        res = _R()
        res.results = [{k: np.array(sim.cores[c].tensor(k))
                        for k in ("out_act", "out_pass")} for c in range(NCORES)]
    else:
        trace = os.environ.get("KERNEL_TRACE") == "1"
        res = run_bass_kernel_spmd(nc, in_maps, list(range(NCORES)), trace=trace)
        if trace:
            import kernel as _k
            _k.LAST_EXEC_NS = res.exec_time_ns
            print(f"HW exec time: {res.exec_time_ns} ns")

    x_hat_member = np.zeros((NM, DM), np.float32)
    x_hat_provider = np.zeros((NP_, DP), np.float32)
    for c in range(NCORES):
        r = res.results[c]
        act = r["out_act"]                     # [NBLK, 128feat, 128dst]
        for b in range(ABLK):
            n0 = b * 128
            n1 = min(n0 + 128, ASH)
            x_hat_member[c * ASH + n0:c * ASH + n1] = act[b][:, :n1 - n0].T
        for b in range(ABLK):
            n0 = b * 128
            n1 = min(n0 + 128, ASH)
            x_hat_provider[c * ASH + n0:c * ASH + n1] = act[ABLK + b][:DP, :n1 - n0].T
        x_hat_member[NACT + c * PSH:NACT + (c + 1) * PSH] = r["out_pass"][:, :PSH].T
    edge_logits = np.zeros(E, np.float32)
    return (x_hat_member, x_hat_provider, edge_logits)


# revision 8
# speedup vs baseline: 2.9756x; 2.9756x over previous
"""Self-contained Trainium2 Bass kernel for nn_GCNAutoencoder_4827543241244.

Replicates reference.py's exact semantics (including jax OOB behavior: scatter
drops, gather clips — see analysis below), distributed over 8 NeuronCores.

True computation (from reference.py with jax OOB semantics):
  nodes 0..119999; member-node k = node k (feature x_member[k]);
  prov-node k = node 100000+k (feature x_provider[k] zero-padded to 128).
  For each input edge e (p_e=provider_idx[e]<20000, m_e=member_idx[e]<100000):
    B-edge (always):          prov-node min(m_e,19999) -> member-node p_e
    A-edge (only m_e<20000):  member-node p_e          -> prov-node m_e
  deg[member-node k<20000] = 1+|{e:p_e=k}|; deg[member-node k>=20000] = 1;
  deg[prov-node k] = 1+|{e:m_e=k, m_e<20000}|... = 1+cnt_m(k).
  GCN layer: out[d] = dinv_d*sum_e dinv_s*(x_s@W) + dinv_d^2*(x_d@W) + b.
  h = relu(out1); z = gcn2(h); x_hat = z@Wdec + bdec.
  Member-nodes >= 20000 are a pure per-row MLP.

Device strategy per core (8-way SPMD):
  - shards: active members [c*2500,(c+1)*2500), prov-nodes likewise -> 40 dst
    blocks of 128; passive members 10000 rows/core (MLP).
  - Segment-sum on TensorE: per 128-edge chunk, gather source rows (dma_gather,
    bf16 256B rows) as stationary lhsT; one-hot S (built by one DVE
    tensor_scalar: (iota==dst_local)*w_src) as moving rhs; PSUM accumulates
    segsum^T [feat, dst]. Self-loop term via a diagonal matmul that also
    initializes PSUM (start=True). All dst-side dinv scaling is deferred (relu
    commutes with positive col-scales; biases handled via K=1 matmuls against
    sqrt(deg)) and applied once at the end.
  - Layer2 shares z'=hpre@W2 (padded bf16 rows, garbage pad cols) across cores
    via two AllGathers; gathers read the AG output directly.
"""
import numpy as np
import ml_dtypes

import concourse.bass as bass
import concourse.bacc as bacc
import concourse.mybir as mybir
import concourse.tile as tile
from concourse.bass_utils import run_bass_kernel_spmd

bfnp = ml_dtypes.bfloat16
BF = mybir.dt.bfloat16
F32 = mybir.dt.float32
I16 = mybir.dt.int16

NM, NP_, E = 100000, 20000, 500000
DM, DP, HID, LAT = 128, 64, 128, 32
NCORES = 8
NACT = 20000
ASH = NACT // NCORES          # 2500
ABLK = 20                     # 128-blocks per shard side
APAD = ABLK * 128             # 2560
NBLK = 2 * ABLK               # 40
SHROWS = 2 * APAD             # 5120
NPASS = NM - NACT             # 80000
PSH = NPASS // NCORES         # 10000
PASSW = 10240                 # padded passive cols per core
MAXCH = 32                    # chunks per dma_gather call (4096 idx)

WIN_B = [(0, 32), (32, 32), (64, 32), (96, 32)]
WIN_A = [(0, 64), (64, 64)]


def _tab2_row(k):
    return (k // ASH) * APAD + (k % ASH)


def _build_sched(per_core, nblocks, windows):
    """per_core: list of 8 (dst_local, src_row, w_src). Returns uniform schedule
    + per-core filled arrays (sr, dl, ws) of shape [nch*128]."""
    nw = len(windows)
    wb_arr = np.array([w[0] for w in windows])
    counts = np.zeros((NCORES, nblocks, nw), np.int64)
    parts = {}
    for c, (dl, sr, ws) in enumerate(per_core):
        blk = dl // 128
        dlb = dl - blk * 128
        wi = np.searchsorted(wb_arr, dlb, side="right") - 1
        key = blk * nw + wi
        order = np.lexsort((dl, key))
        parts[c] = (key[order], dl[order], sr[order], ws[order])
        np.add.at(counts[c], (blk, wi), 1)
    budget = -(-counts.max(axis=0) // 128)          # [nblocks, nw] chunks
    sched = []                                       # (block, wb, ww, nk)
    for b in range(nblocks):
        for w in range(nw):
            if budget[b, w]:
                sched.append((b, windows[w][0], windows[w][1], int(budget[b, w])))
    nch = int(budget.sum())
    core_arrs = []
    for c in range(NCORES):
        key, dl, sr, ws = parts[c]
        DLa = np.full(nch * 128, -1, np.float32)
        SRa = np.zeros(nch * 128, np.int64)
        WSa = np.zeros(nch * 128, np.float32)
        pos = 0
        i = 0
        for b in range(nblocks):
            for w in range(nw):
                nk = budget[b, w]
                if nk == 0:
                    continue
                kk = b * nw + w
                j = i
                while j < len(key) and key[j] == kk:
                    j += 1
                n = j - i
                assert n <= nk * 128
                DLa[pos:pos + n] = dl[i:j] - b * 128
                SRa[pos:pos + n] = sr[i:j]
                WSa[pos:pos + n] = ws[i:j]
                pos += nk * 128
                i = j
        core_arrs.append((DLa, SRa, WSa))
    return sched, nch, core_arrs


def _pack_idx(sr):
    """[nch*128] -> wrapped int16 idx array [128, nch*8]."""
    n = len(sr)
    arr = np.zeros((16, n // 16), np.int16)
    ii = np.arange(n)
    arr[ii % 16, ii // 16] = sr.astype(np.int16)
    return np.tile(arr, (8, 1))


def _preprocess(provider_idx, member_idx):
    p_e = np.asarray(provider_idx).astype(np.int64)
    m_e = np.asarray(member_idx).astype(np.int64)
    cnt_p = np.bincount(p_e, minlength=NACT)[:NACT]
    cnt_m = np.bincount(np.minimum(m_e, NACT), minlength=NACT + 1)[:NACT]
    # deg[prov-node k] counts edges with m_e == k (k<20000) -- min() above would
    # wrongly add m_e>=20000 edges to node 19999; recount properly:
    cnt_m = np.bincount(m_e[m_e < NACT], minlength=NACT)[:NACT]
    dinv_mn = (1.0 + cnt_p) ** -0.5
    dinv_pn = (1.0 + cnt_m) ** -0.5

    mclip = np.minimum(m_e, NACT - 1)
    a_mask = m_e < NACT
    CLIP = NACT - 1
    is_clip = mclip == CLIP          # ~80% of edges: identical source row
    dclip = float(dinv_pn[CLIP])

    coreB, coreA = {1: [], 2: []}, {1: [], 2: []}
    ccols = []
    for c in range(NCORES):
        lo, hi = c * ASH, (c + 1) * ASH
        insh = (p_e >= lo) & (p_e < hi)
        sb = insh & ~is_clip
        coreB[1].append((p_e[sb] - lo, mclip[sb], dinv_pn[mclip[sb]]))
        coreB[2].append((p_e[sb] - lo, _tab2_row(mclip[sb]), dinv_pn[mclip[sb]] ** 2))
        cnt = np.bincount(p_e[insh & is_clip] - lo, minlength=APAD)[:APAD]
        ccols.append((
            (cnt * dclip).astype(np.float32)[None, :],
            (cnt * dclip * dclip).astype(np.float32)[None, :],
        ))
        sa = a_mask & (m_e >= lo) & (m_e < hi)
        coreA[1].append((m_e[sa] - lo, p_e[sa], dinv_mn[p_e[sa]]))
        coreA[2].append((m_e[sa] - lo, _tab2_row(p_e[sa]), dinv_mn[p_e[sa]] ** 2))

    streams = {}
    for L in (1, 2):
        streams[f"B{L}"] = _build_sched(coreB[L], ABLK, WIN_B)
        streams[f"A{L}"] = _build_sched(coreA[L], ABLK, WIN_A)

    per_core_meta = []
    for c in range(NCORES):
        dd = np.zeros(SHROWS, np.float64)
        dd[:ASH] = dinv_mn[c * ASH:(c + 1) * ASH]
        dd[APAD:APAD + ASH] = dinv_pn[c * ASH:(c + 1) * ASH]
        sq = np.zeros(SHROWS, np.float64)
        sq[:ASH] = 1.0 / dinv_mn[c * ASH:(c + 1) * ASH]
        sq[APAD:APAD + ASH] = 1.0 / dinv_pn[c * ASH:(c + 1) * ASH]
        per_core_meta.append(dict(
            ccol1=ccols[c][0], ccol2=ccols[c][1],
            wdst1=dd.reshape(NBLK, 128).T.astype(np.float32).copy(),
            wdst2=(dd ** 2).reshape(NBLK, 128).T.astype(np.float32).copy(),
            dinv_row=dd.astype(np.float32)[None, :].copy(),
            sqrt_row=sq.astype(np.float32)[None, :].copy(),
        ))
    return streams, per_core_meta


def _emit_stream_consts(nc, name, nch):
    """DRAM tensors for one stream's per-chunk metadata + gather indices."""
    if nch == 0:
        return None
    return dict(
        idx=nc.dram_tensor(f"{name}_idx", [128, nch * 8], I16, kind="ExternalInput").ap(),
        dl=nc.dram_tensor(f"{name}_dl", [128, nch], F32, kind="ExternalInput").ap(),
        ws=nc.dram_tensor(f"{name}_ws", [128, nch], F32, kind="ExternalInput").ap(),
    )


def _build_program(streams):
    nc = bacc.Bacc("TRN2", target_bir_lowering=False, debug=False,
                   num_devices=NCORES)
    D = {}
    D["am_tab"] = nc.dram_tensor("am_tab", [NACT, 128], BF, kind="ExternalInput").ap()
    D["pn_tab"] = nc.dram_tensor("pn_tab", [NACT, 128], BF, kind="ExternalInput").ap()
    D["xsh"] = nc.dram_tensor("xsh", [128, NBLK * 128], BF, kind="ExternalInput").ap()
    D["xpassT"] = nc.dram_tensor("xpassT", [128, PASSW], BF, kind="ExternalInput").ap()
    for nm_ in ("W1", "W2", "Wdec", "Wc"):
        shp = [LAT, 128] if nm_ == "Wdec" else [128, LAT] if nm_ == "W2" else [128, 128]
        D[nm_] = nc.dram_tensor(nm_, shp, BF, kind="ExternalInput").ap()
    D["b1"] = nc.dram_tensor("b1", [1, 128], BF, kind="ExternalInput").ap()
    D["cvec"] = nc.dram_tensor("cvec", [1, 128], BF, kind="ExternalInput").ap()
    D["ccol1"] = nc.dram_tensor("ccol1", [1, APAD], F32, kind="ExternalInput").ap()
    D["ccol2"] = nc.dram_tensor("ccol2", [1, APAD], F32, kind="ExternalInput").ap()
    D["rowclip1"] = nc.dram_tensor("rowclip1", [1, 128], F32, kind="ExternalInput").ap()
    D["wdst1"] = nc.dram_tensor("wdst1", [128, NBLK], F32, kind="ExternalInput").ap()
    D["wdst2"] = nc.dram_tensor("wdst2", [128, NBLK], F32, kind="ExternalInput").ap()
    D["dinv_row"] = nc.dram_tensor("dinv_row", [1, SHROWS], F32, kind="ExternalInput").ap()
    D["sqrt_row"] = nc.dram_tensor("sqrt_row", [1, SHROWS], F32, kind="ExternalInput").ap()
    for sname in ("B1", "A1", "B2", "A2"):
        D[sname] = _emit_stream_consts(nc, sname, streams[sname][1])
    D["out_act"] = nc.dram_tensor("out_act", [NBLK, 128, 128], F32, kind="ExternalOutput").ap()
    D["out_pass"] = nc.dram_tensor("out_pass", [128, PASSW], F32, kind="ExternalOutput").ap()

    with tile.TileContext(nc) as tc:
        with (
            tc.tile_pool(name="const", bufs=1) as cpool,
            tc.tile_pool(name="meta", bufs=1) as mpool,
            tc.tile_pool(name="gb", bufs=3) as gpool,
            tc.tile_pool(name="st", bufs=4) as spool,
            tc.tile_pool(name="work", bufs=2) as wpool,
            tc.tile_pool(name="zres", bufs=1) as zpool,
            tc.tile_pool(name="ps", bufs=2, space="PSUM") as ps,
            tc.tile_pool(name="ps2", bufs=2, space="PSUM") as ps2,
            tc.tile_pool(name="psmlp", bufs=2, space="PSUM") as psmlp,
            tc.tile_pool(name="dram", bufs=1, space="DRAM") as dpool,
        ):
            # ---------- constants ----------
            iota = cpool.tile([128, 128], F32)
            nc.gpsimd.iota(iota[:], pattern=[[1, 128]], base=0,
                           channel_multiplier=0, allow_small_or_imprecise_dtypes=True)
            pidx = cpool.tile([128, 1], F32)
            nc.gpsimd.iota(pidx[:], pattern=[[0, 1]], base=0,
                           channel_multiplier=1, allow_small_or_imprecise_dtypes=True)
            ones_row = cpool.tile([1, 512], F32)
            nc.vector.memset(ones_row[:], 1.0)
            ones_bf = cpool.tile([1, 512], BF)
            nc.vector.memset(ones_bf[:], 1.0)
            consts = {}
            for nm_ in ("W1", "W2", "Wdec", "Wc", "b1", "cvec"):
                t = cpool.tile(list(D[nm_].shape), BF, name=f"c_{nm_}")
                nc.sync.dma_start(t[:], D[nm_][:])
                consts[nm_] = t
            wdst1 = cpool.tile([128, NBLK], F32)
            nc.sync.dma_start(wdst1[:], D["wdst1"][:])
            wdst2 = cpool.tile([128, NBLK], F32)
            nc.sync.dma_start(wdst2[:], D["wdst2"][:])
            dinv_row = cpool.tile([1, SHROWS], F32)
            nc.sync.dma_start(dinv_row[:], D["dinv_row"][:])
            sqrt_row = cpool.tile([1, SHROWS], F32)
            nc.sync.dma_start(sqrt_row[:], D["sqrt_row"][:])
            sqrt_bf = cpool.tile([1, SHROWS], BF)
            nc.vector.tensor_copy(sqrt_bf[:], sqrt_row[:])
            ccol1 = cpool.tile([1, APAD], F32)
            nc.sync.dma_start(ccol1[:], D["ccol1"][:])
            ccol2 = cpool.tile([1, APAD], F32)
            nc.sync.dma_start(ccol2[:], D["ccol2"][:])
            rowclip1 = cpool.tile([1, 128], F32)
            nc.sync.dma_start(rowclip1[:], D["rowclip1"][:])
            rowclip2 = cpool.tile([1, 128], F32)

            # resident per-dst-block self rows: x (L1) and z'pad (L2)
            xsh_t = cpool.tile([128, NBLK * 128], BF)   # [dst%128, b*128+feat]
            nc.sync.dma_start(xsh_t[:], D["xsh"][:])
            zself = zpool.tile([128, NBLK * 128], BF)
            nc.gpsimd.memset(zself[:], 0.0)

            # stream metadata resident tiles
            smeta = {}
            for sname in ("B1", "A1", "B2", "A2"):
                if D[sname] is None:
                    continue
                nch = streams[sname][1]
                dlt = mpool.tile([128, nch], F32, name=f"dl_{sname}")
                nc.sync.dma_start(dlt[:], D[sname]["dl"][:])
                wst = mpool.tile([128, nch], F32, name=f"ws_{sname}")
                nc.sync.dma_start(wst[:], D[sname]["ws"][:])
                idxt = mpool.tile([128, nch * 8], I16, name=f"ix_{sname}")
                nc.sync.dma_start(idxt[:], D[sname]["idx"][:])
                smeta[sname] = (dlt, wst, idxt)

            # AG bounce + tables
            ag_in_am = dpool.tile([APAD, 128], BF)
            ag_in_pn = dpool.tile([APAD, 128], BF)
            am_tab2 = dpool.tile([NCORES * APAD, 128], BF, addr_space="Shared")
            pn_tab2 = dpool.tile([NCORES * APAD, 128], BF, addr_space="Shared")

            # ---------- graph layer ----------
            def graph_layer(L, tabB, tabA, wdst, wexp):
                """wexp: function block -> nothing; L=1 computes z', L=2 out."""
                for sname in (f"B{L}", f"A{L}"):
                    pass
                # precompute call partitions per stream
                def calls_of(nch):
                    out = []
                    c0 = 0
                    while c0 < nch:
                        n = min(MAXCH, nch - c0)
                        out.append((c0, n))
                        c0 += n
                    return out

                for side, tabl in (("B", tabB), ("A", tabA)):
                    sname = f"{side}{L}"
                    sched, nch, _ = streams[sname]
                    if nch == 0:
                        continue
                    dlt, wst, idxt = smeta[sname]
                    calls = calls_of(nch)
                    gt = {}
                    # chunk -> (call index, slot)
                    c2call = {}
                    for ci, (c0, n) in enumerate(calls):
                        for k in range(n):
                            c2call[c0 + k] = (ci, k)
                    blk_off = 0 if side == "B" else ABLK
                    # iterate blocks in schedule order
                    chunk_id = 0
                    bcur = -1
                    psum_blk = None
                    gtile = None
                    gci = -1
                    for (b, wb, ww, nk) in sched:
                        gb = b + blk_off
                        if gb != bcur:
                            if bcur >= 0:
                                finish_block(L, bcur, psum_blk)
                            bcur = gb
                            psum_blk = ps.tile([128, 128], F32, name="blkps", tag="blkps")
                            # self-term (also zeroes psum): D = (iota==p)*wdst[:,gb]
                            Dt = spool.tile([128, 128], BF, name="Dt", tag="s")
                            nc.vector.tensor_scalar(
                                out=Dt[:], in0=iota[:], scalar1=pidx[:],
                                scalar2=wdst[:, gb:gb + 1],
                                op0=mybir.AluOpType.is_equal, op1=mybir.AluOpType.mult)
                            selfsrc = xsh_t if L == 1 else zself
                            nc.tensor.matmul(
                                psum_blk[:],
                                selfsrc[:, gb * 128:(gb + 1) * 128],
                                Dt[:], start=True, stop=False, skip_group_check=True)
                            if side == "B":
                                rcl = rowclip1 if L == 1 else rowclip2
                                ccl = ccol1 if L == 1 else ccol2
                                nc.tensor.matmul(
                                    psum_blk[:],
                                    rcl[:],
                                    ccl[:, b * 128:(b + 1) * 128],
                                    start=False, stop=False, skip_group_check=True)
                        for k in range(nk):
                            ci, slot = c2call[chunk_id]
                            if ci != gci:
                                c0, ncall = calls[ci]
                                gtile = gpool.tile([128, MAXCH, 128], BF,
                                                   name="g", tag=f"g{side}")
                                nc.gpsimd.dma_gather(
                                    out_ap=gtile[:, :ncall, :],
                                    in_ap=tabl[:],
                                    idxs_ap=idxt[:, c0 * 8:(c0 + ncall) * 8],
                                    num_idxs=ncall * 128,
                                    num_idxs_reg=ncall * 128,
                                    elem_size=128,
                                    single_packet=False)
                                gci = ci
                            St = spool.tile([128, 128], BF, name="St", tag="s")
                            nc.vector.tensor_scalar(
                                out=St[:, :ww], in0=iota[:, wb:wb + ww],
                                scalar1=dlt[:, chunk_id:chunk_id + 1],
                                scalar2=wst[:, chunk_id:chunk_id + 1],
                                op0=mybir.AluOpType.is_equal, op1=mybir.AluOpType.mult)
                            nc.tensor.matmul(
                                psum_blk[:, wb:wb + ww],
                                gtile[:, slot, :],
                                St[:, :ww], start=False, stop=False,
                                skip_group_check=True)
                            chunk_id += 1
                    if bcur >= 0:
                        finish_block(L, bcur, psum_blk)

            def finish_block(L, gb, psum_blk):
                sl = slice(gb * 128, (gb + 1) * 128)
                if L == 1:
                    P1 = wpool.tile([128, 128], BF, name="P1", tag="p1")
                    nc.scalar.activation(P1[:], psum_blk[:], mybir.ActivationFunctionType.Copy)
                    ps_h = ps2.tile([128, 128], F32, name="psh", tag="psh")
                    nc.tensor.matmul(ps_h[:], consts["W1"][:], P1[:],
                                     start=True, stop=False, skip_group_check=True)
                    nc.tensor.matmul(ps_h[:], consts["b1"][:], sqrt_bf[:, sl],
                                     start=False, stop=True, skip_group_check=True)
                    hpre = wpool.tile([128, 128], BF, name="hpre", tag="hpre")
                    nc.scalar.activation(hpre[:], ps_h[:],
                                         mybir.ActivationFunctionType.Relu)
                    ps_z = ps2.tile([128, LAT], F32, name="psz", tag="psz")
                    nc.tensor.matmul(ps_z[:], hpre[:], consts["W2"][:],
                                     start=True, stop=True, skip_group_check=True)
                    # write z' into zself pad tile (cols 32:128 stale = garbage ok)
                    nc.scalar.activation(zself[:, gb * 128:gb * 128 + LAT], ps_z[:],
                                         mybir.ActivationFunctionType.Copy)
                    # export padded z' rows to AG input
                    dst = ag_in_am if gb < ABLK else ag_in_pn
                    bb = gb if gb < ABLK else gb - ABLK
                    nc.sync.dma_start(
                        dst[bb * 128:(bb + 1) * 128, :],
                        zself[:, gb * 128:(gb + 1) * 128])
                else:
                    P2 = wpool.tile([LAT, 128], BF, name="P2", tag="p2")
                    nc.scalar.activation(P2[:], psum_blk[:LAT, :], mybir.ActivationFunctionType.Copy)
                    ps_x = ps2.tile([128, 128], F32, name="psx", tag="psh")
                    nc.tensor.matmul(ps_x[:], consts["Wdec"][:], P2[:],
                                     start=True, stop=False, skip_group_check=True)
                    nc.tensor.matmul(ps_x[:], consts["cvec"][:], sqrt_bf[:, sl],
                                     start=False, stop=True, skip_group_check=True)
                    ps_d = ps2.tile([128, 128], F32, name="psd", tag="psz")
                    nc.tensor.matmul(ps_d[:], ones_row[:, :128], dinv_row[:, sl],
                                     start=True, stop=True, skip_group_check=True)
                    dfin = wpool.tile([128, 128], F32, name="dfin", tag="p2")
                    nc.scalar.activation(dfin[:], ps_d[:], mybir.ActivationFunctionType.Copy)
                    xo = wpool.tile([128, 128], F32, name="xo", tag="xo")
                    nc.vector.tensor_tensor(out=xo[:], in0=ps_x[:], in1=dfin[:],
                                            op=mybir.AluOpType.mult)
                    nc.sync.dma_start(D["out_act"][gb], xo[:])

            graph_layer(1, D["pn_tab"], D["am_tab"], wdst1, None)

            # AllGathers (z' tables)
            nc.gpsimd.collective_compute(
                "AllGather", mybir.AluOpType.bypass,
                replica_groups=[list(range(NCORES))],
                ins=[ag_in_am.opt()], outs=[am_tab2.opt()])
            nc.gpsimd.collective_compute(
                "AllGather", mybir.AluOpType.bypass,
                replica_groups=[list(range(NCORES))],
                ins=[ag_in_pn.opt()], outs=[pn_tab2.opt()])

            rowclip2_bf = cpool.tile([1, 128], BF)
            nc.sync.dma_start(
                rowclip2_bf[:],
                pn_tab2[(NACT - 1) // ASH * APAD + (NACT - 1) % ASH, None, :])
            nc.vector.tensor_copy(rowclip2[:], rowclip2_bf[:])

            graph_layer(2, pn_tab2, am_tab2, wdst2, None)

            # ---------- passive MLP ----------
            for t0 in range(0, PASSW, 512):
                xt = wpool.tile([128, 512], BF, name="xt", tag="mlpx")
                nc.sync.dma_start(xt[:], D["xpassT"][:, t0:t0 + 512])
                ph = psmlp.tile([128, 512], F32, name="ph", tag="mlph")
                nc.tensor.matmul(ph[:], consts["W1"][:], xt[:],
                                 start=True, stop=False, skip_group_check=True)
                nc.tensor.matmul(ph[:], consts["b1"][:], ones_bf[:],
                                 start=False, stop=True, skip_group_check=True)
                hh = wpool.tile([128, 512], BF, name="hh", tag="mlpx")
                nc.scalar.activation(hh[:], ph[:],
                                     mybir.ActivationFunctionType.Relu)
                po = psmlp.tile([128, 512], F32, name="po", tag="mlph")
                nc.tensor.matmul(po[:], consts["Wc"][:], hh[:],
                                 start=True, stop=False, skip_group_check=True)
                nc.tensor.matmul(po[:], consts["cvec"][:], ones_bf[:],
                                 start=False, stop=True, skip_group_check=True)
                xop = wpool.tile([128, 512], F32, name="xop", tag="mlpo")
                nc.vector.tensor_copy(xop[:], po[:])
                nc.sync.dma_start(D["out_pass"][:, t0:t0 + 512], xop[:])

    nc.compile()
    return nc


def kernel(x_member, x_provider, provider_idx, member_idx,
           W1, b1, W2, b2, Wdec, bdec):
    x_member = np.asarray(x_member, np.float32)
    x_provider = np.asarray(x_provider, np.float32)
    provider_idx = np.asarray(provider_idx)
    member_idx = np.asarray(member_idx)
    W1 = np.asarray(W1, np.float32); b1 = np.asarray(b1, np.float32)
    W2 = np.asarray(W2, np.float32); b2 = np.asarray(b2, np.float32)
    Wdec = np.asarray(Wdec, np.float32); bdec = np.asarray(bdec, np.float32)

    streams, meta = _preprocess(provider_idx, member_idx)
    nc = _build_program(streams)

    xm_bf = x_member.astype(bfnp)
    xp_bf = np.zeros((NP_, 128), bfnp)
    xp_bf[:, :DP] = x_provider.astype(bfnp)
    Wc = (W2 @ Wdec).astype(bfnp)
    cvec = (b2 @ Wdec + bdec).astype(bfnp)[None, :]
    shared = {
        "am_tab": xm_bf[:NACT].copy(),
        "pn_tab": xp_bf,
        "W1": W1.astype(bfnp), "W2": W2.astype(bfnp),
        "Wdec": Wdec.astype(bfnp), "Wc": Wc,
        "b1": b1.astype(bfnp)[None, :], "cvec": cvec,
    }
    in_maps = []
    for c in range(NCORES):
        m = dict(shared)
        xsh = np.zeros((SHROWS, 128), bfnp)
        xsh[:ASH] = xm_bf[c * ASH:(c + 1) * ASH]
        xsh[APAD:APAD + ASH] = xp_bf[c * ASH:(c + 1) * ASH]
        # device layout: [dst%128, block*128+feat]
        m["xsh"] = np.ascontiguousarray(
            xsh.reshape(NBLK, 128, 128).transpose(1, 0, 2).reshape(128, NBLK * 128))
        xpT = np.zeros((128, PASSW), bfnp)
        xpT[:, :PSH] = xm_bf[NACT + c * PSH:NACT + (c + 1) * PSH].T
        m["xpassT"] = xpT
        for k in ("wdst1", "wdst2", "dinv_row", "sqrt_row", "ccol1", "ccol2"):
            m[k] = meta[c][k]
        m["rowclip1"] = xp_bf[NACT - 1].astype(np.float32)[None, :]
        for sname in ("B1", "A1", "B2", "A2"):
            sched, nch, core_arrs = streams[sname]
            if nch == 0:
                continue
            DLa, SRa, WSa = core_arrs[c]
            m[f"{sname}_idx"] = _pack_idx(SRa)
            m[f"{sname}_dl"] = DLa.reshape(nch, 128).T.copy()
            m[f"{sname}_ws"] = WSa.reshape(nch, 128).T.copy()
        in_maps.append(m)

    import os
    if os.environ.get("KERNEL_SIM") == "1":
        import concourse.bass_interp as bass_interp
        sim = bass_interp.MultiCoreSim(nc, NCORES, num_workers=1)
        for c in range(NCORES):
            cs = sim.cores[c]
            for k, v in in_maps[c].items():
                cs.tensor(k)[:] = v
        sim.simulate()
        class _R:
            pass
        res = _R()
        res.results = [{k: np.array(sim.cores[c].tensor(k))
                        for k in ("out_act", "out_pass")} for c in range(NCORES)]
    else:
        trace = os.environ.get("KERNEL_TRACE") == "1"
        res = run_bass_kernel_spmd(nc, in_maps, list(range(NCORES)), trace=trace)
        if trace:
            import kernel as _k
            _k.LAST_EXEC_NS = res.exec_time_ns
            print(f"HW exec time: {res.exec_time_ns} ns")

    x_hat_member = np.zeros((NM, DM), np.float32)
    x_hat_provider = np.zeros((NP_, DP), np.float32)
    for c in range(NCORES):
        r = res.results[c]
        act = r["out_act"]                     # [NBLK, 128feat, 128dst]
        for b in range(ABLK):
            n0 = b * 128
            n1 = min(n0 + 128, ASH)
            x_hat_member[c * ASH + n0:c * ASH + n1] = act[b][:, :n1 - n0].T
        for b in range(ABLK):
            n0 = b * 128
            n1 = min(n0 + 128, ASH)
            x_hat_provider[c * ASH + n0:c * ASH + n1] = act[ABLK + b][:DP, :n1 - n0].T
        x_hat_member[NACT + c * PSH:NACT + (c + 1) * PSH] = r["out_pass"][:, :PSH].T
    edge_logits = np.zeros(E, np.float32)
    return (x_hat_member, x_hat_provider, edge_logits)


# revision 12
# speedup vs baseline: 2.9986x; 1.0077x over previous
"""Self-contained Trainium2 Bass kernel for nn_GCNAutoencoder_4827543241244.

Replicates reference.py's exact semantics (including jax OOB behavior: scatter
drops, gather clips — see analysis below), distributed over 8 NeuronCores.

True computation (from reference.py with jax OOB semantics):
  nodes 0..119999; member-node k = node k (feature x_member[k]);
  prov-node k = node 100000+k (feature x_provider[k] zero-padded to 128).
  For each input edge e (p_e=provider_idx[e]<20000, m_e=member_idx[e]<100000):
    B-edge (always):          prov-node min(m_e,19999) -> member-node p_e
    A-edge (only m_e<20000):  member-node p_e          -> prov-node m_e
  deg[member-node k<20000] = 1+|{e:p_e=k}|; deg[member-node k>=20000] = 1;
  deg[prov-node k] = 1+|{e:m_e=k, m_e<20000}|... = 1+cnt_m(k).
  GCN layer: out[d] = dinv_d*sum_e dinv_s*(x_s@W) + dinv_d^2*(x_d@W) + b.
  h = relu(out1); z = gcn2(h); x_hat = z@Wdec + bdec.
  Member-nodes >= 20000 are a pure per-row MLP.

Device strategy per core (8-way SPMD):
  - shards: active members [c*2500,(c+1)*2500), prov-nodes likewise -> 40 dst
    blocks of 128; passive members 10000 rows/core (MLP).
  - Segment-sum on TensorE: per 128-edge chunk, gather source rows (dma_gather,
    bf16 256B rows) as stationary lhsT; one-hot S (built by one DVE
    tensor_scalar: (iota==dst_local)*w_src) as moving rhs; PSUM accumulates
    segsum^T [feat, dst]. Self-loop term via a diagonal matmul that also
    initializes PSUM (start=True). All dst-side dinv scaling is deferred (relu
    commutes with positive col-scales; biases handled via K=1 matmuls against
    sqrt(deg)) and applied once at the end.
  - Layer2 shares z'=hpre@W2 (padded bf16 rows, garbage pad cols) across cores
    via two AllGathers; gathers read the AG output directly.
"""
import numpy as np
import ml_dtypes

import concourse.bass as bass
import concourse.bacc as bacc
import concourse.mybir as mybir
import concourse.tile as tile
from concourse.bass_utils import run_bass_kernel_spmd

bfnp = ml_dtypes.bfloat16
BF = mybir.dt.bfloat16
F32 = mybir.dt.float32
I16 = mybir.dt.int16

NM, NP_, E = 100000, 20000, 500000
DM, DP, HID, LAT = 128, 64, 128, 32
NCORES = 8
NACT = 20000
ASH = NACT // NCORES          # 2500
ABLK = 20                     # 128-blocks per shard side
APAD = ABLK * 128             # 2560
NBLK = 2 * ABLK               # 40
SHROWS = 2 * APAD             # 5120
NPASS = NM - NACT             # 80000
PSH = NPASS // NCORES         # 10000
PASSW = 10240                 # padded passive cols per core
MAXCH = 32                    # chunks per dma_gather call (4096 idx)

WIN_B = [(0, 32), (32, 32), (64, 32), (96, 32)]
WIN_A = [(0, 64), (64, 64)]


def _tab2_row(k):
    return (k // ASH) * APAD + (k % ASH)


def _build_sched(per_core, nblocks, windows):
    """per_core: list of 8 (dst_local, src_row, w_src). Returns uniform schedule
    + per-core filled arrays (sr, dl, ws) of shape [nch*128]."""
    nw = len(windows)
    wb_arr = np.array([w[0] for w in windows])
    counts = np.zeros((NCORES, nblocks, nw), np.int64)
    parts = {}
    for c, (dl, sr, ws) in enumerate(per_core):
        blk = dl // 128
        dlb = dl - blk * 128
        wi = np.searchsorted(wb_arr, dlb, side="right") - 1
        key = blk * nw + wi
        order = np.lexsort((dl, key))
        parts[c] = (key[order], dl[order], sr[order], ws[order])
        np.add.at(counts[c], (blk, wi), 1)
    budget = -(-counts.max(axis=0) // 128)          # [nblocks, nw] chunks
    sched = []                                       # (block, wb, ww, nk)
    for b in range(nblocks):
        for w in range(nw):
            if budget[b, w]:
                sched.append((b, windows[w][0], windows[w][1], int(budget[b, w])))
    nch = int(budget.sum())
    core_arrs = []
    for c in range(NCORES):
        key, dl, sr, ws = parts[c]
        DLa = np.full(nch * 128, -1, np.float32)
        SRa = np.zeros(nch * 128, np.int64)
        WSa = np.zeros(nch * 128, np.float32)
        pos = 0
        i = 0
        for b in range(nblocks):
            for w in range(nw):
                nk = budget[b, w]
                if nk == 0:
                    continue
                kk = b * nw + w
                j = i
                while j < len(key) and key[j] == kk:
                    j += 1
                n = j - i
                assert n <= nk * 128
                DLa[pos:pos + n] = dl[i:j] - b * 128
                SRa[pos:pos + n] = sr[i:j]
                WSa[pos:pos + n] = ws[i:j]
                pos += nk * 128
                i = j
        core_arrs.append((DLa, SRa, WSa))
    return sched, nch, core_arrs


def _pack_idx(sr):
    """[nch*128] -> wrapped int16 idx array [128, nch*8]."""
    n = len(sr)
    arr = np.zeros((16, n // 16), np.int16)
    ii = np.arange(n)
    arr[ii % 16, ii // 16] = sr.astype(np.int16)
    return np.tile(arr, (8, 1))


def _preprocess(provider_idx, member_idx):
    p_e = np.asarray(provider_idx).astype(np.int64)
    m_e = np.asarray(member_idx).astype(np.int64)
    cnt_p = np.bincount(p_e, minlength=NACT)[:NACT]
    cnt_m = np.bincount(np.minimum(m_e, NACT), minlength=NACT + 1)[:NACT]
    # deg[prov-node k] counts edges with m_e == k (k<20000) -- min() above would
    # wrongly add m_e>=20000 edges to node 19999; recount properly:
    cnt_m = np.bincount(m_e[m_e < NACT], minlength=NACT)[:NACT]
    dinv_mn = (1.0 + cnt_p) ** -0.5
    dinv_pn = (1.0 + cnt_m) ** -0.5

    mclip = np.minimum(m_e, NACT - 1)
    a_mask = m_e < NACT
    CLIP = NACT - 1
    is_clip = mclip == CLIP          # ~80% of edges: identical source row
    dclip = float(dinv_pn[CLIP])

    coreB, coreA = {1: [], 2: []}, {1: [], 2: []}
    ccols = []
    for c in range(NCORES):
        lo, hi = c * ASH, (c + 1) * ASH
        insh = (p_e >= lo) & (p_e < hi)
        sb = insh & ~is_clip
        coreB[1].append((p_e[sb] - lo, mclip[sb], dinv_pn[mclip[sb]]))
        coreB[2].append((p_e[sb] - lo, _tab2_row(mclip[sb]), dinv_pn[mclip[sb]] ** 2))
        cnt = np.bincount(p_e[insh & is_clip] - lo, minlength=APAD)[:APAD]
        ccols.append((
            (cnt * dclip).astype(np.float32)[None, :],
            (cnt * dclip * dclip).astype(np.float32)[None, :],
        ))
        sa = a_mask & (m_e >= lo) & (m_e < hi)
        coreA[1].append((m_e[sa] - lo, p_e[sa], dinv_mn[p_e[sa]]))
        coreA[2].append((m_e[sa] - lo, _tab2_row(p_e[sa]), dinv_mn[p_e[sa]] ** 2))

    streams = {}
    for L in (1, 2):
        streams[f"B{L}"] = _build_sched(coreB[L], ABLK, WIN_B)
        streams[f"A{L}"] = _build_sched(coreA[L], ABLK, WIN_A)

    per_core_meta = []
    for c in range(NCORES):
        dd = np.zeros(SHROWS, np.float64)
        dd[:ASH] = dinv_mn[c * ASH:(c + 1) * ASH]
        dd[APAD:APAD + ASH] = dinv_pn[c * ASH:(c + 1) * ASH]
        sq = np.zeros(SHROWS, np.float64)
        sq[:ASH] = 1.0 / dinv_mn[c * ASH:(c + 1) * ASH]
        sq[APAD:APAD + ASH] = 1.0 / dinv_pn[c * ASH:(c + 1) * ASH]
        per_core_meta.append(dict(
            ccol1=ccols[c][0], ccol2=ccols[c][1],
            wdst1=dd.reshape(NBLK, 128).T.astype(np.float32).copy(),
            wdst2=(dd ** 2).reshape(NBLK, 128).T.astype(np.float32).copy(),
            dinv_row=dd.astype(np.float32)[None, :].copy(),
            sqrt_row=sq.astype(np.float32)[None, :].copy(),
        ))
    return streams, per_core_meta


def _emit_stream_consts(nc, name, nch):
    """DRAM tensors for one stream's per-chunk metadata + gather indices."""
    if nch == 0:
        return None
    return dict(
        idx=nc.dram_tensor(f"{name}_idx", [128, nch * 8], I16, kind="ExternalInput").ap(),
        dl=nc.dram_tensor(f"{name}_dl", [128, nch], F32, kind="ExternalInput").ap(),
        ws=nc.dram_tensor(f"{name}_ws", [128, nch], F32, kind="ExternalInput").ap(),
    )


def _build_program(streams):
    nc = bacc.Bacc("TRN2", target_bir_lowering=False, debug=False,
                   num_devices=NCORES)
    D = {}
    D["am_tab"] = nc.dram_tensor("am_tab", [NACT, 128], BF, kind="ExternalInput").ap()
    D["pn_tab"] = nc.dram_tensor("pn_tab", [NACT, 128], BF, kind="ExternalInput").ap()
    D["xsh"] = nc.dram_tensor("xsh", [128, NBLK * 128], BF, kind="ExternalInput").ap()
    D["xpassT"] = nc.dram_tensor("xpassT", [128, PASSW], BF, kind="ExternalInput").ap()
    for nm_ in ("W1", "W2", "Wdec", "Wc"):
        shp = [LAT, 128] if nm_ == "Wdec" else [128, LAT] if nm_ == "W2" else [128, 128]
        D[nm_] = nc.dram_tensor(nm_, shp, BF, kind="ExternalInput").ap()
    D["b1"] = nc.dram_tensor("b1", [1, 128], BF, kind="ExternalInput").ap()
    D["cvec"] = nc.dram_tensor("cvec", [1, 128], BF, kind="ExternalInput").ap()
    D["ccol1"] = nc.dram_tensor("ccol1", [1, APAD], F32, kind="ExternalInput").ap()
    D["ccol2"] = nc.dram_tensor("ccol2", [1, APAD], F32, kind="ExternalInput").ap()
    D["rowclip1"] = nc.dram_tensor("rowclip1", [1, 128], F32, kind="ExternalInput").ap()
    D["wdst1"] = nc.dram_tensor("wdst1", [128, NBLK], F32, kind="ExternalInput").ap()
    D["wdst2"] = nc.dram_tensor("wdst2", [128, NBLK], F32, kind="ExternalInput").ap()
    D["dinv_row"] = nc.dram_tensor("dinv_row", [1, SHROWS], F32, kind="ExternalInput").ap()
    D["sqrt_bf"] = nc.dram_tensor("sqrt_bf", [1, SHROWS], BF, kind="ExternalInput").ap()
    nch_tot = sum(streams[s][1] for s in ("B1", "A1", "B2", "A2"))
    D["m_idx"] = nc.dram_tensor("m_idx", [128, nch_tot * 8], I16, kind="ExternalInput").ap()
    D["m_dl"] = nc.dram_tensor("m_dl", [128, nch_tot], F32, kind="ExternalInput").ap()
    D["m_ws"] = nc.dram_tensor("m_ws", [128, nch_tot], F32, kind="ExternalInput").ap()
    D["out_act"] = nc.dram_tensor("out_act", [NBLK, 128, 128], F32, kind="ExternalOutput").ap()
    D["out_pass"] = nc.dram_tensor("out_pass", [128, PASSW], F32, kind="ExternalOutput").ap()

    with tile.TileContext(nc) as tc:
        with (
            tc.tile_pool(name="const", bufs=1) as cpool,
            tc.tile_pool(name="meta", bufs=1) as mpool,
            tc.tile_pool(name="gb", bufs=4) as gpool,
            tc.tile_pool(name="ga", bufs=3) as gapool,
            tc.tile_pool(name="st", bufs=4) as spool,
            tc.tile_pool(name="work", bufs=2) as wpool,
            tc.tile_pool(name="zres", bufs=1) as zpool,
            tc.tile_pool(name="ps", bufs=2, space="PSUM") as ps,
            tc.tile_pool(name="ps2", bufs=2, space="PSUM") as ps2,
            tc.tile_pool(name="psmlp", bufs=2, space="PSUM") as psmlp,
            tc.tile_pool(name="dram", bufs=1, space="DRAM") as dpool,
        ):
            # ---------- constants ----------
            iota = cpool.tile([128, 128], F32)
            nc.gpsimd.iota(iota[:], pattern=[[1, 128]], base=0,
                           channel_multiplier=0, allow_small_or_imprecise_dtypes=True)
            pidx = cpool.tile([128, 1], F32)
            nc.gpsimd.iota(pidx[:], pattern=[[0, 1]], base=0,
                           channel_multiplier=1, allow_small_or_imprecise_dtypes=True)
            ones_row = cpool.tile([1, 512], F32)
            nc.vector.memset(ones_row[:], 1.0)
            ones_bf = cpool.tile([1, 512], BF)
            nc.vector.memset(ones_bf[:], 1.0)
            consts = {}
            for nm_ in ("W1", "W2", "Wdec", "Wc", "b1", "cvec"):
                t = cpool.tile(list(D[nm_].shape), BF, name=f"c_{nm_}")
                nc.sync.dma_start(t[:], D[nm_][:])
                consts[nm_] = t
            wdst1 = cpool.tile([128, NBLK], F32)
            nc.sync.dma_start(wdst1[:], D["wdst1"][:])
            wdst2 = cpool.tile([128, NBLK], F32)
            nc.sync.dma_start(wdst2[:], D["wdst2"][:])
            dinv_row = cpool.tile([1, SHROWS], F32)
            nc.sync.dma_start(dinv_row[:], D["dinv_row"][:])
            sqrt_bf = cpool.tile([1, SHROWS], BF)
            nc.sync.dma_start(sqrt_bf[:], D["sqrt_bf"][:])
            ccol1 = cpool.tile([1, APAD], F32)
            nc.sync.dma_start(ccol1[:], D["ccol1"][:])
            ccol2 = cpool.tile([1, APAD], F32)
            nc.sync.dma_start(ccol2[:], D["ccol2"][:])
            rowclip1 = cpool.tile([1, 128], F32)
            nc.sync.dma_start(rowclip1[:], D["rowclip1"][:])
            rowclip2 = cpool.tile([1, 128], F32)

            # resident per-dst-block self rows: x (L1) and z'pad (L2)
            xsh_t = cpool.tile([128, NBLK * 128], BF)   # [dst%128, b*128+feat]
            nc.sync.dma_start(xsh_t[:], D["xsh"][:])
            zself = zpool.tile([128, NBLK * 128], BF)
            nc.gpsimd.memset(zself[:], 0.0)

            # consolidated stream metadata (global chunk offsets)
            dl_all = mpool.tile([128, nch_tot], F32)
            nc.sync.dma_start(dl_all[:], D["m_dl"][:])
            ws_all = mpool.tile([128, nch_tot], F32)
            nc.sync.dma_start(ws_all[:], D["m_ws"][:])
            ix_all = mpool.tile([128, nch_tot * 8], I16)
            nc.sync.dma_start(ix_all[:], D["m_idx"][:])
            wn_all = mpool.tile([128, nch_tot], F32)
            nc.vector.tensor_scalar_mul(wn_all[:], ws_all[:], -1.0)
            soff = {}
            _o = 0
            for sname in ("B1", "A1", "B2", "A2"):
                soff[sname] = _o
                _o += streams[sname][1]

            # AG bounce + tables
            ag_in_am = dpool.tile([APAD, 128], BF)
            ag_in_pn = dpool.tile([APAD, 128], BF)
            am_tab2 = dpool.tile([NCORES * APAD, 128], BF, addr_space="Shared")
            pn_tab2 = dpool.tile([NCORES * APAD, 128], BF, addr_space="Shared")

            # ---------- graph layer ----------
            def graph_layer(L, tabB, tabA, wdst, wexp):
                def calls_of(nch):
                    out = []
                    c0 = 0
                    while c0 < nch:
                        n = min(MAXCH, nch - c0)
                        out.append((c0, n))
                        c0 += n
                    return out

                # issue every gather call of both sides upfront
                gtiles = {}
                callmap = {}
                for side, tabl in (("B", tabB), ("A", tabA)):
                    sname = f"{side}{L}"
                    sched, nch, _ = streams[sname]
                    if nch == 0:
                        continue
                    off = soff[sname]
                    calls = calls_of(nch)
                    pool = gpool if side == "B" else gapool
                    tl = []
                    c2call = {}
                    for ci, (c0, n) in enumerate(calls):
                        gt = pool.tile([128, MAXCH, 128], BF, name="g",
                                       tag=f"g{side}")
                        nc.gpsimd.dma_gather(
                            out_ap=gt[:, :n, :],
                            in_ap=tabl[:],
                            idxs_ap=ix_all[:, (off + c0) * 8:(off + c0 + n) * 8],
                            num_idxs=n * 128,
                            num_idxs_reg=n * 128,
                            elem_size=128,
                            single_packet=False)
                        tl.append(gt)
                        for k in range(n):
                            c2call[c0 + k] = (ci, k)
                    gtiles[side] = tl
                    callmap[side] = c2call

                for side in ("B", "A"):
                    sname = f"{side}{L}"
                    sched, nch, _ = streams[sname]
                    if nch == 0:
                        continue
                    off = soff[sname]
                    blk_off = 0 if side == "B" else ABLK
                    chunk_id = 0
                    bcur = -1
                    psum_blk = None
                    for (b, wb, ww, nk) in sched:
                        gb = b + blk_off
                        if gb != bcur:
                            if bcur >= 0:
                                finish_block(L, bcur, psum_blk)
                            bcur = gb
                            psum_blk = ps.tile([128, 128], F32, name="blkps", tag="blkps")
                            Dt = spool.tile([128, 128], BF, name="Dt", tag="s")
                            nc.vector.tensor_scalar(
                                out=Dt[:], in0=iota[:], scalar1=pidx[:],
                                scalar2=wdst[:, gb:gb + 1],
                                op0=mybir.AluOpType.is_equal, op1=mybir.AluOpType.mult)
                            selfsrc = xsh_t if L == 1 else zself
                            nc.tensor.matmul(
                                psum_blk[:],
                                selfsrc[:, gb * 128:(gb + 1) * 128],
                                Dt[:], start=True, stop=False, skip_group_check=True)
                            if side == "B":
                                rcl = rowclip1 if L == 1 else rowclip2
                                ccl = ccol1 if L == 1 else ccol2
                                nc.tensor.matmul(
                                    psum_blk[:],
                                    rcl[:],
                                    ccl[:, b * 128:(b + 1) * 128],
                                    start=False, stop=False, skip_group_check=True)
                        for k in range(nk):
                            ci, slot = callmap[side][chunk_id]
                            gtile = gtiles[side][ci]
                            if (chunk_id % 2) == 0:
                                St = spool.tile([128, 128], BF, name="St", tag="s")
                                nc.vector.tensor_scalar(
                                    out=St[:, :ww], in0=iota[:, wb:wb + ww],
                                    scalar1=dl_all[:, off + chunk_id:off + chunk_id + 1],
                                    scalar2=ws_all[:, off + chunk_id:off + chunk_id + 1],
                                    op0=mybir.AluOpType.is_equal, op1=mybir.AluOpType.mult)
                            else:
                                tt = spool.tile([128, 128], F32, name="tt", tag="tabs")
                                nc.scalar.activation(
                                    tt[:, :ww], iota[:, wb:wb + ww],
                                    mybir.ActivationFunctionType.Abs,
                                    bias=dl_all[:, off + chunk_id:off + chunk_id + 1], scale=-1.0)
                                St = spool.tile([128, 128], BF, name="St", tag="s")
                                nc.scalar.activation(
                                    St[:, :ww], tt[:, :ww],
                                    mybir.ActivationFunctionType.Relu,
                                    bias=ws_all[:, off + chunk_id:off + chunk_id + 1],
                                    scale=wn_all[:, off + chunk_id:off + chunk_id + 1])
                            nc.tensor.matmul(
                                psum_blk[:, wb:wb + ww],
                                gtile[:, slot, :],
                                St[:, :ww], start=False, stop=False,
                                skip_group_check=True)
                            chunk_id += 1
                    if bcur >= 0:
                        finish_block(L, bcur, psum_blk)

            def finish_block(L, gb, psum_blk):
                sl = slice(gb * 128, (gb + 1) * 128)
                if L == 1:
                    P1 = wpool.tile([128, 128], BF, name="P1", tag="p1")
                    nc.scalar.activation(P1[:], psum_blk[:], mybir.ActivationFunctionType.Copy)
                    ps_h = ps2.tile([128, 128], F32, name="psh", tag="psh")
                    nc.tensor.matmul(ps_h[:], consts["W1"][:], P1[:],
                                     start=True, stop=False, skip_group_check=True)
                    nc.tensor.matmul(ps_h[:], consts["b1"][:], sqrt_bf[:, sl],
                                     start=False, stop=True, skip_group_check=True)
                    hpre = wpool.tile([128, 128], BF, name="hpre", tag="hpre")
                    nc.scalar.activation(hpre[:], ps_h[:],
                                         mybir.ActivationFunctionType.Relu)
                    ps_z = ps2.tile([128, LAT], F32, name="psz", tag="psz")
                    nc.tensor.matmul(ps_z[:], hpre[:], consts["W2"][:],
                                     start=True, stop=True, skip_group_check=True)
                    # write z' into zself pad tile (cols 32:128 stale = garbage ok)
                    nc.scalar.activation(zself[:, gb * 128:gb * 128 + LAT], ps_z[:],
                                         mybir.ActivationFunctionType.Copy)
                    # export padded z' rows to AG input
                    dst = ag_in_am if gb < ABLK else ag_in_pn
                    bb = gb if gb < ABLK else gb - ABLK
                    nc.sync.dma_start(
                        dst[bb * 128:(bb + 1) * 128, :],
                        zself[:, gb * 128:(gb + 1) * 128])
                else:
                    P2 = wpool.tile([LAT, 128], BF, name="P2", tag="p2")
                    nc.scalar.activation(P2[:], psum_blk[:LAT, :], mybir.ActivationFunctionType.Copy)
                    ps_x = ps2.tile([128, 128], F32, name="psx", tag="psh")
                    nc.tensor.matmul(ps_x[:], consts["Wdec"][:], P2[:],
                                     start=True, stop=False, skip_group_check=True)
                    nc.tensor.matmul(ps_x[:], consts["cvec"][:], sqrt_bf[:, sl],
                                     start=False, stop=True, skip_group_check=True)
                    ps_d = ps2.tile([128, 128], F32, name="psd", tag="psz")
                    nc.tensor.matmul(ps_d[:], ones_row[:, :128], dinv_row[:, sl],
                                     start=True, stop=True, skip_group_check=True)
                    dfin = wpool.tile([128, 128], F32, name="dfin", tag="p2")
                    nc.scalar.activation(dfin[:], ps_d[:], mybir.ActivationFunctionType.Copy)
                    xo = wpool.tile([128, 128], F32, name="xo", tag="xo")
                    nc.vector.tensor_tensor(out=xo[:], in0=ps_x[:], in1=dfin[:],
                                            op=mybir.AluOpType.mult)
                    nc.sync.dma_start(D["out_act"][gb], xo[:])

            graph_layer(1, D["pn_tab"], D["am_tab"], wdst1, None)

            # AllGathers (z' tables)
            nc.gpsimd.collective_compute(
                "AllGather", mybir.AluOpType.bypass,
                replica_groups=[list(range(NCORES))],
                ins=[ag_in_am.opt()], outs=[am_tab2.opt()])
            nc.gpsimd.collective_compute(
                "AllGather", mybir.AluOpType.bypass,
                replica_groups=[list(range(NCORES))],
                ins=[ag_in_pn.opt()], outs=[pn_tab2.opt()])

            rowclip2_bf = cpool.tile([1, 128], BF)
            nc.sync.dma_start(
                rowclip2_bf[:],
                pn_tab2[(NACT - 1) // ASH * APAD + (NACT - 1) % ASH, None, :])
            nc.vector.tensor_copy(rowclip2[:], rowclip2_bf[:])

            # ---------- passive MLP ----------
            for t0 in range(0, PASSW, 512):
                xt = wpool.tile([128, 512], BF, name="xt", tag="mlpx")
                nc.sync.dma_start(xt[:], D["xpassT"][:, t0:t0 + 512])
                ph = psmlp.tile([128, 512], F32, name="ph", tag="mlph")
                nc.tensor.matmul(ph[:], consts["W1"][:], xt[:],
                                 start=True, stop=False, skip_group_check=True)
                nc.tensor.matmul(ph[:], consts["b1"][:], ones_bf[:],
                                 start=False, stop=True, skip_group_check=True)
                hh = wpool.tile([128, 512], BF, name="hh", tag="mlpx")
                nc.scalar.activation(hh[:], ph[:],
                                     mybir.ActivationFunctionType.Relu)
                po = psmlp.tile([128, 512], F32, name="po", tag="mlph")
                nc.tensor.matmul(po[:], consts["Wc"][:], hh[:],
                                 start=True, stop=False, skip_group_check=True)
                nc.tensor.matmul(po[:], consts["cvec"][:], ones_bf[:],
                                 start=False, stop=True, skip_group_check=True)
                xop = wpool.tile([128, 512], F32, name="xop", tag="mlpo")
                nc.vector.tensor_copy(xop[:], po[:])
                nc.sync.dma_start(D["out_pass"][:, t0:t0 + 512], xop[:])

            graph_layer(2, pn_tab2, am_tab2, wdst2, None)

    nc.compile()
    return nc


def kernel(x_member, x_provider, provider_idx, member_idx,
           W1, b1, W2, b2, Wdec, bdec):
    x_member = np.asarray(x_member, np.float32)
    x_provider = np.asarray(x_provider, np.float32)
    provider_idx = np.asarray(provider_idx)
    member_idx = np.asarray(member_idx)
    W1 = np.asarray(W1, np.float32); b1 = np.asarray(b1, np.float32)
    W2 = np.asarray(W2, np.float32); b2 = np.asarray(b2, np.float32)
    Wdec = np.asarray(Wdec, np.float32); bdec = np.asarray(bdec, np.float32)

    streams, meta = _preprocess(provider_idx, member_idx)
    nc = _build_program(streams)

    xm_bf = x_member.astype(bfnp)
    xp_bf = np.zeros((NP_, 128), bfnp)
    xp_bf[:, :DP] = x_provider.astype(bfnp)
    Wc = (W2 @ Wdec).astype(bfnp)
    cvec = (b2 @ Wdec + bdec).astype(bfnp)[None, :]
    shared = {
        "am_tab": xm_bf[:NACT].copy(),
        "pn_tab": xp_bf,
        "W1": W1.astype(bfnp), "W2": W2.astype(bfnp),
        "Wdec": Wdec.astype(bfnp), "Wc": Wc,
        "b1": b1.astype(bfnp)[None, :], "cvec": cvec,
    }
    in_maps = []
    for c in range(NCORES):
        m = dict(shared)
        xsh = np.zeros((SHROWS, 128), bfnp)
        xsh[:ASH] = xm_bf[c * ASH:(c + 1) * ASH]
        xsh[APAD:APAD + ASH] = xp_bf[c * ASH:(c + 1) * ASH]
        # device layout: [dst%128, block*128+feat]
        m["xsh"] = np.ascontiguousarray(
            xsh.reshape(NBLK, 128, 128).transpose(1, 0, 2).reshape(128, NBLK * 128))
        xpT = np.zeros((128, PASSW), bfnp)
        xpT[:, :PSH] = xm_bf[NACT + c * PSH:NACT + (c + 1) * PSH].T
        m["xpassT"] = xpT
        for k in ("wdst1", "wdst2", "dinv_row", "ccol1", "ccol2"):
            m[k] = meta[c][k]
        m["sqrt_bf"] = meta[c]["sqrt_row"].astype(bfnp)
        m["rowclip1"] = xp_bf[NACT - 1].astype(np.float32)[None, :]
        DLs, SRs, WSs = [], [], []
        for sname in ("B1", "A1", "B2", "A2"):
            sched, nch, core_arrs = streams[sname]
            DLa, SRa, WSa = core_arrs[c]
            DLs.append(DLa); SRs.append(SRa); WSs.append(WSa)
        DLa = np.concatenate(DLs); SRa = np.concatenate(SRs); WSa = np.concatenate(WSs)
        nch_tot = len(DLa) // 128
        m["m_idx"] = _pack_idx(SRa)
        m["m_dl"] = DLa.reshape(nch_tot, 128).T.copy()
        m["m_ws"] = WSa.reshape(nch_tot, 128).T.copy()
        in_maps.append(m)

    import os
    if os.environ.get("KERNEL_SIM") == "1":
        import concourse.bass_interp as bass_interp
        sim = bass_interp.MultiCoreSim(nc, NCORES, num_workers=1)
        for c in range(NCORES):
            cs = sim.cores[c]
            for k, v in in_maps[c].items():
                cs.tensor(k)[:] = v
        sim.simulate()
        class _R:
            pass
        res = _R()
        res.results = [{k: np.array(sim.cores[c].tensor(k))
                        for k in ("out_act", "out_pass")} for c in range(NCORES)]
    else:
        trace = os.environ.get("KERNEL_TRACE") == "1"
        res = run_bass_kernel_spmd(nc, in_maps, list(range(NCORES)), trace=trace)
        if trace:
            import kernel as _k
            _k.LAST_EXEC_NS = res.exec_time_ns
            print(f"HW exec time: {res.exec_time_ns} ns")

    x_hat_member = np.zeros((NM, DM), np.float32)
    x_hat_provider = np.zeros((NP_, DP), np.float32)
    for c in range(NCORES):
        r = res.results[c]
        act = r["out_act"]                     # [NBLK, 128feat, 128dst]
        for b in range(ABLK):
            n0 = b * 128
            n1 = min(n0 + 128, ASH)
            x_hat_member[c * ASH + n0:c * ASH + n1] = act[b][:, :n1 - n0].T
        for b in range(ABLK):
            n0 = b * 128
            n1 = min(n0 + 128, ASH)
            x_hat_provider[c * ASH + n0:c * ASH + n1] = act[ABLK + b][:DP, :n1 - n0].T
        x_hat_member[NACT + c * PSH:NACT + (c + 1) * PSH] = r["out_pass"][:, :PSH].T
    edge_logits = np.zeros(E, np.float32)
    return (x_hat_member, x_hat_provider, edge_logits)


# revision 13
# speedup vs baseline: 3.1654x; 1.0556x over previous
"""Self-contained Trainium2 Bass kernel for nn_GCNAutoencoder_4827543241244.

Replicates reference.py's exact semantics (including jax OOB behavior: scatter
drops, gather clips — see analysis below), distributed over 8 NeuronCores.

True computation (from reference.py with jax OOB semantics):
  nodes 0..119999; member-node k = node k (feature x_member[k]);
  prov-node k = node 100000+k (feature x_provider[k] zero-padded to 128).
  For each input edge e (p_e=provider_idx[e]<20000, m_e=member_idx[e]<100000):
    B-edge (always):          prov-node min(m_e,19999) -> member-node p_e
    A-edge (only m_e<20000):  member-node p_e          -> prov-node m_e
  deg[member-node k<20000] = 1+|{e:p_e=k}|; deg[member-node k>=20000] = 1;
  deg[prov-node k] = 1+|{e:m_e=k, m_e<20000}|... = 1+cnt_m(k).
  GCN layer: out[d] = dinv_d*sum_e dinv_s*(x_s@W) + dinv_d^2*(x_d@W) + b.
  h = relu(out1); z = gcn2(h); x_hat = z@Wdec + bdec.
  Member-nodes >= 20000 are a pure per-row MLP.

Device strategy per core (8-way SPMD):
  - shards: active members [c*2500,(c+1)*2500), prov-nodes likewise -> 40 dst
    blocks of 128; passive members 10000 rows/core (MLP).
  - Segment-sum on TensorE: per 128-edge chunk, gather source rows (dma_gather,
    bf16 256B rows) as stationary lhsT; one-hot S (built by one DVE
    tensor_scalar: (iota==dst_local)*w_src) as moving rhs; PSUM accumulates
    segsum^T [feat, dst]. Self-loop term via a diagonal matmul that also
    initializes PSUM (start=True). All dst-side dinv scaling is deferred (relu
    commutes with positive col-scales; biases handled via K=1 matmuls against
    sqrt(deg)) and applied once at the end.
  - Layer2 shares z'=hpre@W2 (padded bf16 rows, garbage pad cols) across cores
    via two AllGathers; gathers read the AG output directly.
"""
import numpy as np
import ml_dtypes

import concourse.bass as bass
import concourse.bacc as bacc
import concourse.mybir as mybir
import concourse.tile as tile
from concourse.bass_utils import run_bass_kernel_spmd

bfnp = ml_dtypes.bfloat16
BF = mybir.dt.bfloat16
F32 = mybir.dt.float32
I16 = mybir.dt.int16

NM, NP_, E = 100000, 20000, 500000
DM, DP, HID, LAT = 128, 64, 128, 32
NCORES = 8
NACT = 20000
ASH = NACT // NCORES          # 2500
ABLK = 20                     # 128-blocks per shard side
APAD = ABLK * 128             # 2560
NBLK = 2 * ABLK               # 40
SHROWS = 2 * APAD             # 5120
NPASS = NM - NACT             # 80000
PSH = NPASS // NCORES         # 10000
PASSW = 10240                 # padded passive cols per core
MAXCH = 32                    # chunks per dma_gather call (4096 idx)

WIN_B = [(0, 64), (64, 64)]
WIN_A = [(0, 64), (64, 64)]


def _tab2_row(k):
    return (k // ASH) * APAD + (k % ASH)


def _build_sched(per_core, nblocks, windows):
    """per_core: list of 8 (dst_local, src_row, w_src). Returns uniform schedule
    + per-core filled arrays (sr, dl, ws) of shape [nch*128]."""
    nw = len(windows)
    wb_arr = np.array([w[0] for w in windows])
    counts = np.zeros((NCORES, nblocks, nw), np.int64)
    parts = {}
    for c, (dl, sr, ws) in enumerate(per_core):
        blk = dl // 128
        dlb = dl - blk * 128
        wi = np.searchsorted(wb_arr, dlb, side="right") - 1
        key = blk * nw + wi
        order = np.lexsort((dl, key))
        parts[c] = (key[order], dl[order], sr[order], ws[order])
        np.add.at(counts[c], (blk, wi), 1)
    budget = -(-counts.max(axis=0) // 128)          # [nblocks, nw] chunks
    sched = []                                       # (block, wb, ww, nk)
    for b in range(nblocks):
        for w in range(nw):
            if budget[b, w]:
                sched.append((b, windows[w][0], windows[w][1], int(budget[b, w])))
    nch = int(budget.sum())
    core_arrs = []
    for c in range(NCORES):
        key, dl, sr, ws = parts[c]
        DLa = np.full(nch * 128, -1, np.float32)
        SRa = np.zeros(nch * 128, np.int64)
        WSa = np.zeros(nch * 128, np.float32)
        pos = 0
        i = 0
        for b in range(nblocks):
            for w in range(nw):
                nk = budget[b, w]
                if nk == 0:
                    continue
                kk = b * nw + w
                j = i
                while j < len(key) and key[j] == kk:
                    j += 1
                n = j - i
                assert n <= nk * 128
                DLa[pos:pos + n] = dl[i:j] - b * 128
                SRa[pos:pos + n] = sr[i:j]
                WSa[pos:pos + n] = ws[i:j]
                pos += nk * 128
                i = j
        core_arrs.append((DLa, SRa, WSa))
    return sched, nch, core_arrs


def _pack_idx(sr):
    """[nch*128] -> wrapped int16 idx array [128, nch*8]."""
    n = len(sr)
    arr = np.zeros((16, n // 16), np.int16)
    ii = np.arange(n)
    arr[ii % 16, ii // 16] = sr.astype(np.int16)
    return np.tile(arr, (8, 1))


def _preprocess(provider_idx, member_idx):
    p_e = np.asarray(provider_idx).astype(np.int64)
    m_e = np.asarray(member_idx).astype(np.int64)
    cnt_p = np.bincount(p_e, minlength=NACT)[:NACT]
    cnt_m = np.bincount(np.minimum(m_e, NACT), minlength=NACT + 1)[:NACT]
    # deg[prov-node k] counts edges with m_e == k (k<20000) -- min() above would
    # wrongly add m_e>=20000 edges to node 19999; recount properly:
    cnt_m = np.bincount(m_e[m_e < NACT], minlength=NACT)[:NACT]
    dinv_mn = (1.0 + cnt_p) ** -0.5
    dinv_pn = (1.0 + cnt_m) ** -0.5

    mclip = np.minimum(m_e, NACT - 1)
    a_mask = m_e < NACT
    CLIP = NACT - 1
    is_clip = mclip == CLIP          # ~80% of edges: identical source row
    dclip = float(dinv_pn[CLIP])

    coreB, coreA = {1: [], 2: []}, {1: [], 2: []}
    ccols = []
    for c in range(NCORES):
        lo, hi = c * ASH, (c + 1) * ASH
        insh = (p_e >= lo) & (p_e < hi)
        sb = insh & ~is_clip
        coreB[1].append((p_e[sb] - lo, mclip[sb], dinv_pn[mclip[sb]]))
        coreB[2].append((p_e[sb] - lo, _tab2_row(mclip[sb]), dinv_pn[mclip[sb]] ** 2))
        cnt = np.bincount(p_e[insh & is_clip] - lo, minlength=APAD)[:APAD]
        ccols.append((
            (cnt * dclip).astype(np.float32)[None, :],
            (cnt * dclip * dclip).astype(np.float32)[None, :],
        ))
        sa = a_mask & (m_e >= lo) & (m_e < hi)
        coreA[1].append((m_e[sa] - lo, p_e[sa], dinv_mn[p_e[sa]]))
        coreA[2].append((m_e[sa] - lo, _tab2_row(p_e[sa]), dinv_mn[p_e[sa]] ** 2))

    streams = {}
    for L in (1, 2):
        streams[f"B{L}"] = _build_sched(coreB[L], ABLK, WIN_B)
        streams[f"A{L}"] = _build_sched(coreA[L], ABLK, WIN_A)

    per_core_meta = []
    for c in range(NCORES):
        dd = np.zeros(SHROWS, np.float64)
        dd[:ASH] = dinv_mn[c * ASH:(c + 1) * ASH]
        dd[APAD:APAD + ASH] = dinv_pn[c * ASH:(c + 1) * ASH]
        sq = np.zeros(SHROWS, np.float64)
        sq[:ASH] = 1.0 / dinv_mn[c * ASH:(c + 1) * ASH]
        sq[APAD:APAD + ASH] = 1.0 / dinv_pn[c * ASH:(c + 1) * ASH]
        per_core_meta.append(dict(
            ccol1=ccols[c][0], ccol2=ccols[c][1],
            wdst1=dd.reshape(NBLK, 128).T.astype(np.float32).copy(),
            wdst2=(dd ** 2).reshape(NBLK, 128).T.astype(np.float32).copy(),
            dinv_row=dd.astype(np.float32)[None, :].copy(),
            sqrt_row=sq.astype(np.float32)[None, :].copy(),
        ))
    return streams, per_core_meta


def _emit_stream_consts(nc, name, nch):
    """DRAM tensors for one stream's per-chunk metadata + gather indices."""
    if nch == 0:
        return None
    return dict(
        idx=nc.dram_tensor(f"{name}_idx", [128, nch * 8], I16, kind="ExternalInput").ap(),
        dl=nc.dram_tensor(f"{name}_dl", [128, nch], F32, kind="ExternalInput").ap(),
        ws=nc.dram_tensor(f"{name}_ws", [128, nch], F32, kind="ExternalInput").ap(),
    )


def _build_program(streams):
    nc = bacc.Bacc("TRN2", target_bir_lowering=False, debug=False,
                   num_devices=NCORES)
    D = {}
    D["am_tab"] = nc.dram_tensor("am_tab", [NACT, 128], BF, kind="ExternalInput").ap()
    D["pn_tab"] = nc.dram_tensor("pn_tab", [NACT, 128], BF, kind="ExternalInput").ap()
    D["xsh"] = nc.dram_tensor("xsh", [128, NBLK * 128], BF, kind="ExternalInput").ap()
    D["xpassT"] = nc.dram_tensor("xpassT", [128, PASSW], BF, kind="ExternalInput").ap()
    for nm_ in ("W1", "W2", "Wdec", "Wc"):
        shp = [LAT, 128] if nm_ == "Wdec" else [128, LAT] if nm_ == "W2" else [128, 128]
        D[nm_] = nc.dram_tensor(nm_, shp, BF, kind="ExternalInput").ap()
    D["b1"] = nc.dram_tensor("b1", [1, 128], BF, kind="ExternalInput").ap()
    D["cvec"] = nc.dram_tensor("cvec", [1, 128], BF, kind="ExternalInput").ap()
    D["ccol1"] = nc.dram_tensor("ccol1", [1, APAD], F32, kind="ExternalInput").ap()
    D["ccol2"] = nc.dram_tensor("ccol2", [1, APAD], F32, kind="ExternalInput").ap()
    D["rowclip1"] = nc.dram_tensor("rowclip1", [1, 128], F32, kind="ExternalInput").ap()
    D["wdst1"] = nc.dram_tensor("wdst1", [128, NBLK], F32, kind="ExternalInput").ap()
    D["wdst2"] = nc.dram_tensor("wdst2", [128, NBLK], F32, kind="ExternalInput").ap()
    D["dinv_row"] = nc.dram_tensor("dinv_row", [1, SHROWS], F32, kind="ExternalInput").ap()
    D["sqrt_bf"] = nc.dram_tensor("sqrt_bf", [1, SHROWS], BF, kind="ExternalInput").ap()
    nch_tot = sum(streams[s][1] for s in ("B1", "A1", "B2", "A2"))
    D["m_idx"] = nc.dram_tensor("m_idx", [128, nch_tot * 8], I16, kind="ExternalInput").ap()
    D["m_dl"] = nc.dram_tensor("m_dl", [128, nch_tot], F32, kind="ExternalInput").ap()
    D["m_ws"] = nc.dram_tensor("m_ws", [128, nch_tot], F32, kind="ExternalInput").ap()
    D["out_act"] = nc.dram_tensor("out_act", [NBLK, 128, 128], F32, kind="ExternalOutput").ap()
    D["out_pass"] = nc.dram_tensor("out_pass", [128, PASSW], F32, kind="ExternalOutput").ap()

    with tile.TileContext(nc) as tc:
        with (
            tc.tile_pool(name="const", bufs=1) as cpool,
            tc.tile_pool(name="meta", bufs=1) as mpool,
            tc.tile_pool(name="gb", bufs=5) as gpool,
            tc.tile_pool(name="ga", bufs=5) as gapool,
            tc.tile_pool(name="st", bufs=4) as spool,
            tc.tile_pool(name="work", bufs=2) as wpool,
            tc.tile_pool(name="zres", bufs=1) as zpool,
            tc.tile_pool(name="ps", bufs=2, space="PSUM") as ps,
            tc.tile_pool(name="ps2", bufs=2, space="PSUM") as ps2,
            tc.tile_pool(name="psmlp", bufs=2, space="PSUM") as psmlp,
            tc.tile_pool(name="dram", bufs=1, space="DRAM") as dpool,
        ):
            # ---------- constants ----------
            iota = cpool.tile([128, 128], F32)
            nc.gpsimd.iota(iota[:], pattern=[[1, 128]], base=0,
                           channel_multiplier=0, allow_small_or_imprecise_dtypes=True)
            pidx = cpool.tile([128, 1], F32)
            nc.gpsimd.iota(pidx[:], pattern=[[0, 1]], base=0,
                           channel_multiplier=1, allow_small_or_imprecise_dtypes=True)
            ones_row = cpool.tile([1, 512], F32)
            nc.vector.memset(ones_row[:], 1.0)
            ones_bf = cpool.tile([1, 512], BF)
            nc.vector.memset(ones_bf[:], 1.0)
            consts = {}
            for nm_ in ("W1", "W2", "Wdec", "Wc", "b1", "cvec"):
                t = cpool.tile(list(D[nm_].shape), BF, name=f"c_{nm_}")
                nc.sync.dma_start(t[:], D[nm_][:])
                consts[nm_] = t
            wdst1 = cpool.tile([128, NBLK], F32)
            nc.sync.dma_start(wdst1[:], D["wdst1"][:])
            wdst2 = cpool.tile([128, NBLK], F32)
            nc.sync.dma_start(wdst2[:], D["wdst2"][:])
            dinv_row = cpool.tile([1, SHROWS], F32)
            nc.sync.dma_start(dinv_row[:], D["dinv_row"][:])
            sqrt_bf = cpool.tile([1, SHROWS], BF)
            nc.sync.dma_start(sqrt_bf[:], D["sqrt_bf"][:])
            ccol1 = cpool.tile([1, APAD], F32)
            nc.sync.dma_start(ccol1[:], D["ccol1"][:])
            ccol2 = cpool.tile([1, APAD], F32)
            nc.sync.dma_start(ccol2[:], D["ccol2"][:])
            rowclip1 = cpool.tile([1, 128], F32)
            nc.sync.dma_start(rowclip1[:], D["rowclip1"][:])
            rowclip2 = cpool.tile([1, 128], F32)

            # resident per-dst-block self rows: x (L1) and z'pad (L2)
            xsh_t = cpool.tile([128, NBLK * 128], BF)   # [dst%128, b*128+feat]
            nc.sync.dma_start(xsh_t[:], D["xsh"][:])
            zself = zpool.tile([128, NBLK * 128], BF)
            nc.gpsimd.memset(zself[:], 0.0)

            # consolidated stream metadata (global chunk offsets)
            dl_all = mpool.tile([128, nch_tot], F32)
            nc.sync.dma_start(dl_all[:], D["m_dl"][:])
            ws_all = mpool.tile([128, nch_tot], F32)
            nc.sync.dma_start(ws_all[:], D["m_ws"][:])
            ix_all = mpool.tile([128, nch_tot * 8], I16)
            nc.sync.dma_start(ix_all[:], D["m_idx"][:])
            wn_all = mpool.tile([128, nch_tot], F32)
            nc.vector.tensor_scalar_mul(wn_all[:], ws_all[:], -1.0)
            soff = {}
            _o = 0
            for sname in ("B1", "A1", "B2", "A2"):
                soff[sname] = _o
                _o += streams[sname][1]

            # AG bounce + tables
            ag_in = dpool.tile([SHROWS, 128], BF)
            ag_out = dpool.tile([NCORES * SHROWS, 128], BF, addr_space="Shared")
            am_tab2 = dpool.tile([NCORES * APAD, 128], BF)
            pn_tab2 = dpool.tile([NCORES * APAD, 128], BF)

            # ---------- graph layer ----------
            def graph_layer(L, tabB, tabA, wdst, wexp):
                def calls_of(nch):
                    out = []
                    c0 = 0
                    while c0 < nch:
                        n = min(MAXCH, nch - c0)
                        out.append((c0, n))
                        c0 += n
                    return out

                # issue every gather call of both sides upfront
                gtiles = {}
                callmap = {}
                for side, tabl in (("B", tabB), ("A", tabA)):
                    sname = f"{side}{L}"
                    sched, nch, _ = streams[sname]
                    if nch == 0:
                        continue
                    off = soff[sname]
                    calls = calls_of(nch)
                    pool = gpool if side == "B" else gapool
                    tl = []
                    c2call = {}
                    for ci, (c0, n) in enumerate(calls):
                        gt = pool.tile([128, MAXCH, 128], BF, name="g",
                                       tag=f"g{side}")
                        nc.gpsimd.dma_gather(
                            out_ap=gt[:, :n, :],
                            in_ap=tabl[:],
                            idxs_ap=ix_all[:, (off + c0) * 8:(off + c0 + n) * 8],
                            num_idxs=n * 128,
                            num_idxs_reg=n * 128,
                            elem_size=128,
                            single_packet=False)
                        tl.append(gt)
                        for k in range(n):
                            c2call[c0 + k] = (ci, k)
                    gtiles[side] = tl
                    callmap[side] = c2call

                for side in ("B", "A"):
                    sname = f"{side}{L}"
                    sched, nch, _ = streams[sname]
                    if nch == 0:
                        continue
                    off = soff[sname]
                    blk_off = 0 if side == "B" else ABLK
                    chunk_id = 0
                    bcur = -1
                    psum_blk = None
                    for (b, wb, ww, nk) in sched:
                        gb = b + blk_off
                        if gb != bcur:
                            if bcur >= 0:
                                finish_block(L, bcur, psum_blk)
                            bcur = gb
                            psum_blk = ps.tile([128, 128], F32, name="blkps", tag="blkps")
                            Dt = spool.tile([128, 128], BF, name="Dt", tag="s")
                            nc.vector.tensor_scalar(
                                out=Dt[:], in0=iota[:], scalar1=pidx[:],
                                scalar2=wdst[:, gb:gb + 1],
                                op0=mybir.AluOpType.is_equal, op1=mybir.AluOpType.mult)
                            selfsrc = xsh_t if L == 1 else zself
                            nc.tensor.matmul(
                                psum_blk[:],
                                selfsrc[:, gb * 128:(gb + 1) * 128],
                                Dt[:], start=True, stop=False, skip_group_check=True)
                            if side == "B":
                                rcl = rowclip1 if L == 1 else rowclip2
                                ccl = ccol1 if L == 1 else ccol2
                                nc.tensor.matmul(
                                    psum_blk[:],
                                    rcl[:],
                                    ccl[:, b * 128:(b + 1) * 128],
                                    start=False, stop=False, skip_group_check=True)
                        for k in range(nk):
                            ci, slot = callmap[side][chunk_id]
                            gtile = gtiles[side][ci]
                            if (chunk_id % 2) == 0:
                                St = spool.tile([128, 128], BF, name="St", tag="s")
                                nc.vector.tensor_scalar(
                                    out=St[:, :ww], in0=iota[:, wb:wb + ww],
                                    scalar1=dl_all[:, off + chunk_id:off + chunk_id + 1],
                                    scalar2=ws_all[:, off + chunk_id:off + chunk_id + 1],
                                    op0=mybir.AluOpType.is_equal, op1=mybir.AluOpType.mult)
                            else:
                                tt = spool.tile([128, 128], F32, name="tt", tag="tabs")
                                nc.scalar.activation(
                                    tt[:, :ww], iota[:, wb:wb + ww],
                                    mybir.ActivationFunctionType.Abs,
                                    bias=dl_all[:, off + chunk_id:off + chunk_id + 1], scale=-1.0)
                                St = spool.tile([128, 128], BF, name="St", tag="s")
                                nc.scalar.activation(
                                    St[:, :ww], tt[:, :ww],
                                    mybir.ActivationFunctionType.Relu,
                                    bias=ws_all[:, off + chunk_id:off + chunk_id + 1],
                                    scale=wn_all[:, off + chunk_id:off + chunk_id + 1])
                            nc.tensor.matmul(
                                psum_blk[:, wb:wb + ww],
                                gtile[:, slot, :],
                                St[:, :ww], start=False, stop=False,
                                skip_group_check=True)
                            chunk_id += 1
                    if bcur >= 0:
                        finish_block(L, bcur, psum_blk)

            def finish_block(L, gb, psum_blk):
                sl = slice(gb * 128, (gb + 1) * 128)
                if L == 1:
                    P1 = wpool.tile([128, 128], BF, name="P1", tag="p1")
                    nc.scalar.activation(P1[:], psum_blk[:], mybir.ActivationFunctionType.Copy)
                    ps_h = ps2.tile([128, 128], F32, name="psh", tag="psh")
                    nc.tensor.matmul(ps_h[:], consts["W1"][:], P1[:],
                                     start=True, stop=False, skip_group_check=True)
                    nc.tensor.matmul(ps_h[:], consts["b1"][:], sqrt_bf[:, sl],
                                     start=False, stop=True, skip_group_check=True)
                    hpre = wpool.tile([128, 128], BF, name="hpre", tag="hpre")
                    nc.scalar.activation(hpre[:], ps_h[:],
                                         mybir.ActivationFunctionType.Relu)
                    ps_z = ps2.tile([128, LAT], F32, name="psz", tag="psz")
                    nc.tensor.matmul(ps_z[:], hpre[:], consts["W2"][:],
                                     start=True, stop=True, skip_group_check=True)
                    # write z' into zself pad tile (cols 32:128 stale = garbage ok)
                    nc.scalar.activation(zself[:, gb * 128:gb * 128 + LAT], ps_z[:],
                                         mybir.ActivationFunctionType.Copy)
                    # export padded z' rows to AG input
                    nc.sync.dma_start(
                        ag_in[gb * 128:(gb + 1) * 128, :],
                        zself[:, gb * 128:(gb + 1) * 128])
                else:
                    P2 = wpool.tile([LAT, 128], BF, name="P2", tag="p2")
                    nc.scalar.activation(P2[:], psum_blk[:LAT, :], mybir.ActivationFunctionType.Copy)
                    ps_x = ps2.tile([128, 128], F32, name="psx", tag="psh")
                    nc.tensor.matmul(ps_x[:], consts["Wdec"][:], P2[:],
                                     start=True, stop=False, skip_group_check=True)
                    nc.tensor.matmul(ps_x[:], consts["cvec"][:], sqrt_bf[:, sl],
                                     start=False, stop=True, skip_group_check=True)
                    ps_d = ps2.tile([128, 128], F32, name="psd", tag="psz")
                    nc.tensor.matmul(ps_d[:], ones_row[:, :128], dinv_row[:, sl],
                                     start=True, stop=True, skip_group_check=True)
                    dfin = wpool.tile([128, 128], F32, name="dfin", tag="p2")
                    nc.scalar.activation(dfin[:], ps_d[:], mybir.ActivationFunctionType.Copy)
                    xo = wpool.tile([128, 128], F32, name="xo", tag="xo")
                    nc.vector.tensor_tensor(out=xo[:], in0=ps_x[:], in1=dfin[:],
                                            op=mybir.AluOpType.mult)
                    nc.sync.dma_start(D["out_act"][gb], xo[:])

            graph_layer(1, D["pn_tab"], D["am_tab"], wdst1, None)

            # One AllGather, then split into the two gather tables
            nc.gpsimd.collective_compute(
                "AllGather", mybir.AluOpType.bypass,
                replica_groups=[list(range(NCORES))],
                ins=[ag_in.opt()], outs=[ag_out.opt()])
            ago = ag_out[:].rearrange("(k r) f -> k r f", r=SHROWS)
            nc.sync.dma_start(
                am_tab2[:].rearrange("(k r) f -> k r f", r=APAD),
                ago[:, :APAD, :])
            nc.sync.dma_start(
                pn_tab2[:].rearrange("(k r) f -> k r f", r=APAD),
                ago[:, APAD:, :])

            rowclip2_bf = cpool.tile([1, 128], BF)
            nc.sync.dma_start(
                rowclip2_bf[:],
                pn_tab2[(NACT - 1) // ASH * APAD + (NACT - 1) % ASH, None, :])
            nc.vector.tensor_copy(rowclip2[:], rowclip2_bf[:])

            # ---------- passive MLP ----------
            for t0 in range(0, PASSW, 512):
                xt = wpool.tile([128, 512], BF, name="xt", tag="mlpx")
                nc.sync.dma_start(xt[:], D["xpassT"][:, t0:t0 + 512])
                ph = psmlp.tile([128, 512], F32, name="ph", tag="mlph")
                nc.tensor.matmul(ph[:], consts["W1"][:], xt[:],
                                 start=True, stop=False, skip_group_check=True)
                nc.tensor.matmul(ph[:], consts["b1"][:], ones_bf[:],
                                 start=False, stop=True, skip_group_check=True)
                hh = wpool.tile([128, 512], BF, name="hh", tag="mlpx")
                nc.scalar.activation(hh[:], ph[:],
                                     mybir.ActivationFunctionType.Relu)
                po = psmlp.tile([128, 512], F32, name="po", tag="mlph")
                nc.tensor.matmul(po[:], consts["Wc"][:], hh[:],
                                 start=True, stop=False, skip_group_check=True)
                nc.tensor.matmul(po[:], consts["cvec"][:], ones_bf[:],
                                 start=False, stop=True, skip_group_check=True)
                xop = wpool.tile([128, 512], F32, name="xop", tag="mlpo")
                nc.vector.tensor_copy(xop[:], po[:])
                nc.sync.dma_start(D["out_pass"][:, t0:t0 + 512], xop[:])

            graph_layer(2, pn_tab2, am_tab2, wdst2, None)

    nc.compile()
    return nc


def kernel(x_member, x_provider, provider_idx, member_idx,
           W1, b1, W2, b2, Wdec, bdec):
    x_member = np.asarray(x_member, np.float32)
    x_provider = np.asarray(x_provider, np.float32)
    provider_idx = np.asarray(provider_idx)
    member_idx = np.asarray(member_idx)
    W1 = np.asarray(W1, np.float32); b1 = np.asarray(b1, np.float32)
    W2 = np.asarray(W2, np.float32); b2 = np.asarray(b2, np.float32)
    Wdec = np.asarray(Wdec, np.float32); bdec = np.asarray(bdec, np.float32)

    streams, meta = _preprocess(provider_idx, member_idx)
    nc = _build_program(streams)

    xm_bf = x_member.astype(bfnp)
    xp_bf = np.zeros((NP_, 128), bfnp)
    xp_bf[:, :DP] = x_provider.astype(bfnp)
    Wc = (W2 @ Wdec).astype(bfnp)
    cvec = (b2 @ Wdec + bdec).astype(bfnp)[None, :]
    shared = {
        "am_tab": xm_bf[:NACT].copy(),
        "pn_tab": xp_bf,
        "W1": W1.astype(bfnp), "W2": W2.astype(bfnp),
        "Wdec": Wdec.astype(bfnp), "Wc": Wc,
        "b1": b1.astype(bfnp)[None, :], "cvec": cvec,
    }
    in_maps = []
    for c in range(NCORES):
        m = dict(shared)
        xsh = np.zeros((SHROWS, 128), bfnp)
        xsh[:ASH] = xm_bf[c * ASH:(c + 1) * ASH]
        xsh[APAD:APAD + ASH] = xp_bf[c * ASH:(c + 1) * ASH]
        # device layout: [dst%128, block*128+feat]
        m["xsh"] = np.ascontiguousarray(
            xsh.reshape(NBLK, 128, 128).transpose(1, 0, 2).reshape(128, NBLK * 128))
        xpT = np.zeros((128, PASSW), bfnp)
        xpT[:, :PSH] = xm_bf[NACT + c * PSH:NACT + (c + 1) * PSH].T
        m["xpassT"] = xpT
        for k in ("wdst1", "wdst2", "dinv_row", "ccol1", "ccol2"):
            m[k] = meta[c][k]
        m["sqrt_bf"] = meta[c]["sqrt_row"].astype(bfnp)
        m["rowclip1"] = xp_bf[NACT - 1].astype(np.float32)[None, :]
        DLs, SRs, WSs = [], [], []
        for sname in ("B1", "A1", "B2", "A2"):
            sched, nch, core_arrs = streams[sname]
            DLa, SRa, WSa = core_arrs[c]
            DLs.append(DLa); SRs.append(SRa); WSs.append(WSa)
        DLa = np.concatenate(DLs); SRa = np.concatenate(SRs); WSa = np.concatenate(WSs)
        nch_tot = len(DLa) // 128
        m["m_idx"] = _pack_idx(SRa)
        m["m_dl"] = DLa.reshape(nch_tot, 128).T.copy()
        m["m_ws"] = WSa.reshape(nch_tot, 128).T.copy()
        in_maps.append(m)

    import os
    if os.environ.get("KERNEL_SIM") == "1":
        import concourse.bass_interp as bass_interp
        sim = bass_interp.MultiCoreSim(nc, NCORES, num_workers=1)
        for c in range(NCORES):
            cs = sim.cores[c]
            for k, v in in_maps[c].items():
                cs.tensor(k)[:] = v
        sim.simulate()
        class _R:
            pass
        res = _R()
        res.results = [{k: np.array(sim.cores[c].tensor(k))
                        for k in ("out_act", "out_pass")} for c in range(NCORES)]
    else:
        trace = os.environ.get("KERNEL_TRACE") == "1"
        res = run_bass_kernel_spmd(nc, in_maps, list(range(NCORES)), trace=trace)
        if trace:
            import kernel as _k
            _k.LAST_EXEC_NS = res.exec_time_ns
            print(f"HW exec time: {res.exec_time_ns} ns")

    x_hat_member = np.zeros((NM, DM), np.float32)
    x_hat_provider = np.zeros((NP_, DP), np.float32)
    for c in range(NCORES):
        r = res.results[c]
        act = r["out_act"]                     # [NBLK, 128feat, 128dst]
        for b in range(ABLK):
            n0 = b * 128
            n1 = min(n0 + 128, ASH)
            x_hat_member[c * ASH + n0:c * ASH + n1] = act[b][:, :n1 - n0].T
        for b in range(ABLK):
            n0 = b * 128
            n1 = min(n0 + 128, ASH)
            x_hat_provider[c * ASH + n0:c * ASH + n1] = act[ABLK + b][:DP, :n1 - n0].T
        x_hat_member[NACT + c * PSH:NACT + (c + 1) * PSH] = r["out_pass"][:, :PSH].T
    edge_logits = np.zeros(E, np.float32)
    return (x_hat_member, x_hat_provider, edge_logits)
